# revision 1
# baseline (speedup 1.0000x reference)
"""Trainium2 Bass kernel for nn_ConvmambaProj (bidirectional mamba + dilated-conv branch).

Sharding: 8 cores = (batch b, direction dir) for the mamba scan path, plus
(batch bc, L-half) for the conv branch. Zero cross-core communication; host
does flips/transposes/partial-sum assembly.
"""
import sys

sys.path.insert(0, "/opt/trn_rl_repo")
import numpy as np
import concourse.bass as bass
import concourse.mybir as mybir
from concourse import tile
from concourse.bass_utils import run_bass_kernel_spmd

dt = mybir.dt
AF = mybir.ActivationFunctionType
ALU = mybir.AluOpType

B, L, DM, DI, DS, DR, DC = 4, 2304, 512, 1024, 16, 32, 4
LH = L // 2          # 1152, scan half
NC8 = 8
CEXT = LH + 8        # conv-branch window width (halo 4 each side)
F32, F16 = dt.float32, dt.float16


def _nchunks(total, step=512):
    out = []
    o = 0
    while o < total:
        out.append((o, min(step, total - o)))
        o += step
    return out


def split_sync_waits(nc, max_waits=1):
    for f in nc.m.functions:
        for blk in f.blocks:
            new_insts = []
            for inst in blk.instructions:
                si = getattr(inst, "sync_info", None)
                if si and si.on_wait and len(si.on_wait) > max_waits:
                    extra, keep = si.on_wait[:-max_waits], si.on_wait[-max_waits:]
                    for w in extra:
                        new_insts.append(
                            mybir.InstNoOp(
                                name=nc.get_next_instruction_name(),
                                ins=[],
                                outs=[],
                                sync_info=mybir.SyncInfo(on_wait=[w], on_update=[]),
                                engine=inst.engine,
                            )
                        )
                    inst.sync_info = mybir.SyncInfo(on_wait=keep, on_update=si.on_update)
                new_insts.append(inst)
            blk.instructions = new_insts


def build_nc():
    nc = bass.Bass()

    # ---- external inputs (per core) ----
    hT = nc.dram_tensor("hT", [DM, L], F16, kind="ExternalInput")          # hidden[b].T (flipped if bwd)
    hTc = nc.dram_tensor("hTc", [DM, CEXT], F16, kind="ExternalInput")     # conv window of hidden[bc].T
    mask = nc.dram_tensor("mask", [1, CEXT], F16, kind="ExternalInput")
    w1T = nc.dram_tensor("w1T", [128, 4, 2 * DI], F16, kind="ExternalInput")
    wxz1T = nc.dram_tensor("wxz1T", [128, 4, DI], F16, kind="ExternalInput")
    cw = nc.dram_tensor("cw", [128, 32], F32, kind="ExternalInput")
    cbias = nc.dram_tensor("cbias", [128, 8], F32, kind="ExternalInput")
    xpT = nc.dram_tensor("xpT", [128, 8, 64], F16, kind="ExternalInput")
    selT = nc.dram_tensor("selT", [32, 2 * DS * 128], F16, kind="ExternalInput")
    dpT = nc.dram_tensor("dpT", [DR, DI], F16, kind="ExternalInput")
    dpb = nc.dram_tensor("dpb", [128, 8], F32, kind="ExternalInput")
    Asb = nc.dram_tensor("Asb", [128, 128], F32, kind="ExternalInput")
    Dsb = nc.dram_tensor("Dsb", [128, 8], F32, kind="ExternalInput")
    wopT = nc.dram_tensor("wopT", [128, 8, DM], F16, kind="ExternalInput")
    ident = nc.dram_tensor("ident", [128, 128], F16, kind="ExternalInput")
    phi_i = nc.dram_tensor("phi_i", [128, 4], F32, kind="ExternalInput")
    prew = nc.dram_tensor("prew", [128, 12], F32, kind="ExternalInput")
    preb = nc.dram_tensor("preb", [128, 12], F32, kind="ExternalInput")
    dilw = nc.dram_tensor("dilw", [128, 36], F32, kind="ExternalInput")
    dilb = nc.dram_tensor("dilb", [128, 12], F32, kind="ExternalInput")
    locw = nc.dram_tensor("locw", [128, 12], F32, kind="ExternalInput")
    locb = nc.dram_tensor("locb", [128, 4], F32, kind="ExternalInput")
    lng = nc.dram_tensor("lng", [128, 16], F32, kind="ExternalInput")
    lnb = nc.dram_tensor("lnb", [128, 16], F32, kind="ExternalInput")
    mcombT = nc.dram_tensor("mcombT", [128, 16, DM], F16, kind="ExternalInput")

    # ---- outputs ----
    o_scan = nc.dram_tensor("o_scan", [DM, L], F32, kind="ExternalOutput")
    o_conv = nc.dram_tensor("o_conv", [DM, LH], F32, kind="ExternalOutput")

    # ---- internal DRAM scratch ----
    zbuf = nc.dram_tensor("zbuf", [8, 128, L], F16)
    xbuf = nc.dram_tensor("xbuf", [8, 128, L], F16)
    dbuf = nc.dram_tensor("dbuf", [8, 128, L], F16)   # delta
    ubuf = nc.dram_tensor("ubuf", [8, 128, L], F16)   # du = delta*x

    with tile.TileContext(nc) as tc:
        with (
            tc.tile_pool(name="pc", bufs=1) as pc,
            tc.tile_pool(name="pps", bufs=1, space="PSUM") as pps,
            tc.tile_pool(name="ppy", bufs=1, space="PSUM") as ppy,
        ):
            # persistent small weights
            cw_t = pc.tile([128, 32], F32, tag="cw"); nc.sync.dma_start(cw_t[:], cw[:])
            cb_t = pc.tile([128, 8], F32, tag="cb"); nc.sync.dma_start(cb_t[:], cbias[:])
            xpT_t = pc.tile([128, 8, 64], F16, tag="xpT"); nc.sync.dma_start(xpT_t[:], xpT[:])
            selT_t = pc.tile([32, 2 * DS * 128], F16, tag="selT"); nc.sync.dma_start(selT_t[:], selT[:])
            dpT_t = pc.tile([DR, DI], F16, tag="dpT"); nc.sync.dma_start(dpT_t[:], dpT[:])
            dpb_t = pc.tile([128, 8], F32, tag="dpb"); nc.sync.dma_start(dpb_t[:], dpb[:])
            Asb_t = pc.tile([128, 128], F32, tag="Asb"); nc.sync.dma_start(Asb_t[:], Asb[:])
            Dsb_t = pc.tile([128, 8], F32, tag="Dsb"); nc.sync.dma_start(Dsb_t[:], Dsb[:])
            wopT_t = pc.tile([128, 8, DM], F16, tag="wopT"); nc.sync.dma_start(wopT_t[:], wopT[:])
            id_t = pc.tile([128, 128], F16, tag="ident"); nc.sync.dma_start(id_t[:], ident[:])
            xdbl_sb = pc.tile([64, L], F16, tag="xdbl")
            xbc_sb = pc.tile([2 * DS, L], F16, tag="xbc")

            # ============ Phase A: in_proj + conv1d + silu ============
            with (
                tc.tile_pool(name="pa", bufs=1) as pa,
                tc.tile_pool(name="pxp", bufs=3) as pxp,
                tc.tile_pool(name="px", bufs=8) as px,
                tc.tile_pool(name="pzt", bufs=2) as pzt,
            ):
                hT_t = pa.tile([128, 4, L], F16, tag="hT")
                for k in range(4):
                    nc.sync.dma_start(hT_t[:, k, :], hT[k * 128:(k + 1) * 128, :])
                w1T_t = pa.tile([128, 4, 2 * DI], F16, tag="w1T")
                nc.sync.dma_start(w1T_t[:], w1T[:])

                x_tiles = []
                for m in range(16):  # 0-7: x channels, 8-15: z channels
                    if m < 8:
                        xp_t = pxp.tile([128, 3 + L], F16, tag="xpre")
                        nc.gpsimd.memset(xp_t[:, 0:3], 0.0)
                    for half in range(2):
                        ps = pps.tile([128, 1184], F32, tag="mm")
                        for (off, n) in _nchunks(LH):
                            for k in range(4):
                                nc.tensor.matmul(
                                    ps[:, off:off + n],
                                    w1T_t[:, k, m * 128:(m + 1) * 128],
                                    hT_t[:, k, half * LH + off:half * LH + off + n],
                                    start=(k == 0), stop=(k == 3),
                                )
                        if m < 8:
                            nc.vector.tensor_copy(xp_t[:, 3 + half * LH:3 + (half + 1) * LH], ps[:, 0:LH])
                        else:
                            z_t = pzt.tile([128, LH], F16, tag="zt")
                            nc.scalar.activation(z_t[:], ps[:, 0:LH], AF.Silu)
                            nc.sync.dma_start(zbuf[m - 8, :, half * LH:(half + 1) * LH], z_t[:])
                    if m < 8:
                        # causal depthwise conv (k=4) + bias + silu
                        cv = pzt.tile([128, L], F16, tag="cv")
                        nc.vector.tensor_scalar(cv[:], xp_t[:, 0:L], cw_t[:, m * 4:m * 4 + 1], None, ALU.mult)
                        for j in range(1, 4):
                            nc.vector.scalar_tensor_tensor(
                                cv[:], xp_t[:, j:j + L], cw_t[:, m * 4 + j:m * 4 + j + 1], cv[:],
                                ALU.mult, ALU.add)
                        x_t = px.tile([128, L], F16, tag="x")
                        nc.scalar.activation(x_t[:], cv[:], AF.Silu, bias=cb_t[:, m:m + 1])
                        nc.sync.dma_start(xbuf[m, :, :], x_t[:])
                        x_tiles.append(x_t)

                # ============ Phase B: x_proj, delta, du ============
                for half in range(2):
                    ps = pps.tile([64, 1184], F32, tag="mm")
                    for (off, n) in _nchunks(LH):
                        for k in range(8):
                            nc.tensor.matmul(
                                ps[:, off:off + n],
                                xpT_t[:, k, :],
                                x_tiles[k][:, half * LH + off:half * LH + off + n],
                                start=(k == 0), stop=(k == 7),
                            )
                    nc.scalar.copy(xdbl_sb[:, half * LH:(half + 1) * LH], ps[0:64, 0:LH])
                # B/C rows to a partition-0-based tile (compute engines cannot
                # shift partitions; DMA can)
                nc.sync.dma_start(xbc_sb[:], xdbl_sb[32:64, :])

                for c in range(8):
                    dl_t = pzt.tile([128, L], F16, tag="dl")
                    for half in range(2):
                        ps = pps.tile([128, 1184], F32, tag="mm")
                        for (off, n) in _nchunks(LH):
                            nc.tensor.matmul(
                                ps[:, off:off + n],
                                dpT_t[:, c * 128:(c + 1) * 128],
                                xdbl_sb[0:DR, half * LH + off:half * LH + off + n],
                                start=True, stop=True,
                            )
                        # softplus(x) = ln(exp(x) + 1): Softplus has no ACT table here
                        et = pzt.tile([128, LH], F32, tag="et")
                        nc.scalar.activation(et[:], ps[:, 0:LH], AF.Exp, bias=dpb_t[:, c:c + 1])
                        nc.scalar.activation(dl_t[:, half * LH:(half + 1) * LH], et[:],
                                             AF.Ln, bias=1.0)
                    nc.sync.dma_start(dbuf[c, :, :], dl_t[:])
                    du_t = pzt.tile([128, L], F16, tag="du")
                    nc.vector.tensor_mul(du_t[:], dl_t[:], x_tiles[c][:])
                    nc.sync.dma_start(ubuf[c, :, :], du_t[:])

            # ============ Phase D: conv branch ============
            with (
                tc.tile_pool(name="pd1", bufs=1) as pd1,
                tc.tile_pool(name="pd4", bufs=4) as pd4,
                tc.tile_pool(name="pd16", bufs=16) as pd16,
                tc.tile_pool(name="pdt", bufs=2) as pdt,
            ):
                hTc_t = pd1.tile([128, 4, CEXT], F16, tag="hTc")
                for k in range(4):
                    nc.sync.dma_start(hTc_t[:, k, :], hTc[k * 128:(k + 1) * 128, :])
                wxz1T_t = pd1.tile([128, 4, DI], F16, tag="wxz1T")
                nc.sync.dma_start(wxz1T_t[:], wxz1T[:])
                mcombT_t = pd1.tile([128, 16, DM], F16, tag="mcombT")
                nc.sync.dma_start(mcombT_t[:], mcombT[:])
                prew_t = pd1.tile([128, 12], F32, tag="prew"); nc.sync.dma_start(prew_t[:], prew[:])
                preb_t = pd1.tile([128, 12], F32, tag="preb"); nc.sync.dma_start(preb_t[:], preb[:])
                dilw_t = pd1.tile([128, 36], F32, tag="dilw"); nc.sync.dma_start(dilw_t[:], dilw[:])
                dilb_t = pd1.tile([128, 12], F32, tag="dilb"); nc.sync.dma_start(dilb_t[:], dilb[:])
                locw_t = pd1.tile([128, 12], F32, tag="locw"); nc.sync.dma_start(locw_t[:], locw[:])
                locb_t = pd1.tile([128, 4], F32, tag="locb"); nc.sync.dma_start(locb_t[:], locb[:])
                lng_t = pd1.tile([128, 16], F32, tag="lng"); nc.sync.dma_start(lng_t[:], lng[:])
                lnb_t = pd1.tile([128, 16], F32, tag="lnb"); nc.sync.dma_start(lnb_t[:], lnb[:])
                phi_t = pd1.tile([128, 4], F32, tag="phi"); nc.sync.dma_start(phi_t[:], phi_i[:])
                mask_t = pd1.tile([1, CEXT], F16, tag="mask"); nc.sync.dma_start(mask_t[:], mask[:])
                one1_t = pd1.tile([1, 128], F16, tag="one1")
                nc.gpsimd.memset(one1_t[:], 1.0)
                ones_t = pd1.tile([128, 1], F16, tag="ones")
                nc.gpsimd.memset(ones_t[:], 1.0)

                # mask replicated to 128 partitions
                psm = pps.tile([128, 1184], F32, tag="mm")
                for (off, n) in _nchunks(CEXT):
                    nc.tensor.matmul(psm[:, off:off + n], one1_t[:], mask_t[:, off:off + n],
                                     start=True, stop=True)
                mrep_t = pd1.tile([128, CEXT], F16, tag="mrep")
                nc.scalar.copy(mrep_t[:], psm[:, 0:CEXT])

                # xz1 = in_proj[4096:5120] @ hidden_window ; m 0-3: xa, 4-7: xc
                xa_tiles, xc_tiles = [], []
                for m in range(8):
                    ps = pps.tile([128, 1184], F32, tag="mm")
                    for (off, n) in _nchunks(CEXT):
                        for k in range(4):
                            nc.tensor.matmul(
                                ps[:, off:off + n],
                                wxz1T_t[:, k, m * 128:(m + 1) * 128],
                                hTc_t[:, k, off:off + n],
                                start=(k == 0), stop=(k == 3),
                            )
                    t = pd4.tile([128, CEXT], F16, tag=("xa" if m < 4 else "xcm"))
                    if m < 4:
                        nc.vector.tensor_copy(t[:], ps[:, 0:CEXT])
                        xa_tiles.append(t)
                    else:
                        # xc masked (zero outside valid seq) for conv input
                        nc.vector.tensor_mul(t[:], ps[:, 0:CEXT], mrep_t[:])
                        xc_tiles.append(t)

                cat_tiles = []
                # feats: 3 dilations x 4 ch-tiles (cat channels 0..1535)
                for i, d in enumerate((1, 2, 4)):
                    for t4 in range(4):
                        xp2 = pdt.tile([128, CEXT], F16, tag="xp2")
                        nc.vector.tensor_scalar(xp2[:], xa_tiles[t4][:],
                                                prew_t[:, i * 4 + t4:i * 4 + t4 + 1],
                                                preb_t[:, i * 4 + t4:i * 4 + t4 + 1],
                                                ALU.mult, ALU.add)
                        xpm = pdt.tile([128, CEXT], F16, tag="xpm")
                        nc.vector.tensor_mul(xpm[:], xp2[:], mrep_t[:])
                        ct = pd16.tile([128, LH], F16, tag="cat")
                        base = (i * 4 + t4) * 3
                        nc.vector.tensor_scalar(ct[:], xpm[:, 4 - d:4 - d + LH],
                                                dilw_t[:, base:base + 1], None, ALU.mult)
                        for j in (1, 2):
                            nc.vector.scalar_tensor_tensor(
                                ct[:], xpm[:, 4 - d + j * d:4 - d + j * d + LH],
                                dilw_t[:, base + j:base + j + 1], ct[:], ALU.mult, ALU.add)
                        nc.vector.tensor_scalar(ct[:], ct[:], dilb_t[:, i * 4 + t4:i * 4 + t4 + 1],
                                                None, ALU.add)
                        cat_tiles.append(ct)
                # phi * gelu(local conv + b)  (cat channels 1536..2047)
                for t4 in range(4):
                    lc = pdt.tile([128, LH], F16, tag="lc")
                    nc.vector.tensor_scalar(lc[:], xc_tiles[t4][:, 3:3 + LH],
                                            locw_t[:, t4 * 3:t4 * 3 + 1], None, ALU.mult)
                    for j in (1, 2):
                        nc.vector.scalar_tensor_tensor(
                            lc[:], xc_tiles[t4][:, 3 + j:3 + j + LH],
                            locw_t[:, t4 * 3 + j:t4 * 3 + j + 1], lc[:], ALU.mult, ALU.add)
                    lg = pdt.tile([128, LH], F16, tag="lg")
                    nc.scalar.activation(lg[:], lc[:], AF.Gelu, bias=locb_t[:, t4:t4 + 1])
                    ct = pd16.tile([128, LH], F16, tag="cat")
                    nc.vector.tensor_scalar(ct[:], lg[:], phi_t[:, t4:t4 + 1], None, ALU.mult)
                    cat_tiles.append(ct)

                # LayerNorm over the 2048 channels (partition-dim stats via PE)
                mu = pd1.tile([1, LH], F32, tag="mu")
                pstat = pps.tile([1, 1184], F32, tag="mm")
                for t16 in range(16):
                    for (off, n) in _nchunks(LH):
                        nc.tensor.matmul(pstat[0:1, off:off + n], ones_t[:],
                                         cat_tiles[t16][:, off:off + n],
                                         start=(t16 == 0), stop=(t16 == 15),
                                         skip_group_check=True)
                nc.scalar.activation(mu[:], pstat[0:1, 0:LH], AF.Copy, scale=1.0 / 2048)
                ex2 = pd1.tile([1, LH], F32, tag="ex2")
                pstat2 = pps.tile([1, 1184], F32, tag="mm")
                for t16 in range(16):
                    sq = pdt.tile([128, LH], F16, tag="sq")
                    nc.vector.tensor_mul(sq[:], cat_tiles[t16][:], cat_tiles[t16][:])
                    for (off, n) in _nchunks(LH):
                        nc.tensor.matmul(pstat2[0:1, off:off + n], ones_t[:], sq[:, off:off + n],
                                         start=(t16 == 0), stop=(t16 == 15),
                                         skip_group_check=True)
                nc.scalar.activation(ex2[:], pstat2[0:1, 0:LH], AF.Copy, scale=1.0 / 2048)
                var = pd1.tile([1, LH], F32, tag="var")
                nc.vector.tensor_mul(var[:], mu[:], mu[:])
                nc.vector.tensor_sub(var[:], ex2[:], var[:])
                nc.vector.tensor_scalar_add(var[:], var[:], 1e-5)
                sd = pd1.tile([1, LH], F32, tag="sd")
                nc.scalar.activation(sd[:], var[:], AF.Sqrt)
                rstd = pd1.tile([1, LH], F32, tag="rstd")
                nc.vector.reciprocal(rstd[:], sd[:])
                # replicate mu/rstd to 128 partitions
                one1f = pd1.tile([1, 128], F32, tag="one1f")
                nc.gpsimd.memset(one1f[:], 1.0)
                murep = pd1.tile([128, LH], F16, tag="murep")
                ps1 = pps.tile([128, 1184], F32, tag="mm")
                for (off, n) in _nchunks(LH):
                    nc.tensor.matmul(ps1[:, off:off + n], one1f[:], mu[:, off:off + n],
                                     start=True, stop=True)
                nc.scalar.copy(murep[:], ps1[:, 0:LH])
                rsrep = pd1.tile([128, LH], F16, tag="rsrep")
                ps2 = pps.tile([128, 1184], F32, tag="mm")
                for (off, n) in _nchunks(LH):
                    nc.tensor.matmul(ps2[:, off:off + n], one1f[:], rstd[:, off:off + n],
                                     start=True, stop=True)
                nc.scalar.copy(rsrep[:], ps2[:, 0:LH])

                for t16 in range(16):
                    ct = cat_tiles[t16]
                    nc.vector.tensor_sub(ct[:], ct[:], murep[:])
                    nc.vector.tensor_mul(ct[:], ct[:], rsrep[:])
                    nc.vector.tensor_scalar(ct[:], ct[:], lng_t[:, t16:t16 + 1],
                                            lnb_t[:, t16:t16 + 1], ALU.mult, ALU.add)

                # fused (out_proj[:,2048:] @ cb_fuse_w) @ LN(cat)
                for m in range(4):
                    psf = ppy.tile([128, L], F32, tag="py")
                    for (off, n) in _nchunks(LH):
                        for k in range(16):
                            nc.tensor.matmul(
                                psf[:, off:off + n],
                                mcombT_t[:, k, m * 128:(m + 1) * 128],
                                cat_tiles[k][:, off:off + n],
                                start=(k == 0), stop=(k == 15),
                            )
                    oc = pdt.tile([128, LH], F32, tag="oc")
                    nc.scalar.copy(oc[:], psf[:, 0:LH])
                    nc.sync.dma_start(o_conv[m * 128:(m + 1) * 128, :], oc[:])

            # ============ Phase C: selective scan ============
            with (
                tc.tile_pool(name="pb16", bufs=16) as pb16,
                tc.tile_pool(name="ph1", bufs=1) as ph1,
                tc.tile_pool(name="ps2p", bufs=2) as ps2p,
                tc.tile_pool(name="ps3p", bufs=3) as ps3p,
                tc.tile_pool(name="phl", bufs=8) as phl,
            ):
                hlast = [phl.tile([128, DS], F32, tag="hlast", name=f"hlast{i}")
                         for i in range(8)]
                yg_t = ph1.tile([128, 8, LH], F16, tag="yg")
                for half in range(2):
                    off_h = half * LH
                    # build replicated B/C rows for all 16 states
                    breps, creps = [], []
                    for n in range(DS):
                        for is_c in range(2):
                            psr = pps.tile([128, 1184], F32, tag="mm")
                            for (off, nn) in _nchunks(LH):
                                nc.tensor.matmul(
                                    psr[:, off:off + nn],
                                    selT_t[:, is_c * DS * 128 + n * 128:(is_c * DS + n + 1) * 128],
                                    xbc_sb[:, off_h + off:off_h + off + nn],
                                    start=True, stop=True,
                                )
                            rt = pb16.tile([128, LH], F16, tag=("crep" if is_c else "brep"))
                            if (n + is_c) % 2 == 0:
                                nc.scalar.copy(rt[:], psr[:, 0:LH])
                            else:
                                nc.vector.tensor_copy(rt[:], psr[:, 0:LH])
                            (creps if is_c else breps).append(rt)

                    for c in range(8):
                        dl_t = ps2p.tile([128, LH], F16, tag="dls")
                        nc.sync.dma_start(dl_t[:], dbuf[c, :, off_h:off_h + LH])
                        du_t = ps2p.tile([128, LH], F16, tag="dus")
                        nc.sync.dma_start(du_t[:], ubuf[c, :, off_h:off_h + LH])
                        x_t = ps2p.tile([128, LH], F16, tag="xs")
                        nc.sync.dma_start(x_t[:], xbuf[c, :, off_h:off_h + LH])
                        sz_t = ps2p.tile([128, LH], F16, tag="szs")
                        nc.sync.dma_start(sz_t[:], zbuf[c, :, off_h:off_h + LH])

                        hb = ph1.tile([128, DS, LH], F16, tag="hb")
                        psy = ppy.tile([128, L], F32, tag="py")
                        for n in range(DS):
                            dA = ps3p.tile([128, LH], F16, tag="dA")
                            nc.scalar.activation(dA[:], dl_t[:], AF.Exp,
                                                 scale=Asb_t[:, c * DS + n:c * DS + n + 1])
                            dBu = ps3p.tile([128, LH], F16, tag="dBu")
                            nc.vector.tensor_mul(dBu[:], du_t[:], breps[n][:])
                            init = 0.0 if half == 0 else hlast[c][:, n:n + 1]
                            nc.vector.tensor_tensor_scan(hb[:, n, :], dA[:], dBu[:], init,
                                                         ALU.mult, ALU.add)
                            hC = ps3p.tile([128, LH], F16, tag="hC")
                            nc.vector.tensor_mul(hC[:], hb[:, n, :], creps[n][:])
                            for (off, nn) in _nchunks(LH):
                                nc.tensor.matmul(psy[:, off:off + nn], id_t[:], hC[:, off:off + nn],
                                                 start=(n == 0), stop=(n == DS - 1),
                                                 skip_group_check=True)
                        if half == 0:
                            nc.vector.tensor_copy(hlast[c][:, :], hb[:, :, LH - 1])
                        # epilogue: yg = (y + D*x) * silu(z)
                        tmp = ps2p.tile([128, LH], F16, tag="tmp")
                        nc.vector.scalar_tensor_tensor(tmp[:], x_t[:], Dsb_t[:, c:c + 1],
                                                       psy[:, 0:LH], ALU.mult, ALU.add)
                        nc.vector.tensor_mul(yg_t[:, c, :], tmp[:], sz_t[:])

                    # out_proj partial for this half
                    for m in range(4):
                        pso = ppy.tile([128, L], F32, tag="py")
                        for (off, nn) in _nchunks(LH):
                            for c in range(8):
                                nc.tensor.matmul(
                                    pso[:, off:off + nn],
                                    wopT_t[:, c, m * 128:(m + 1) * 128],
                                    yg_t[:, c, off:off + nn],
                                    start=(c == 0), stop=(c == 7),
                                )
                        ot = ps2p.tile([128, LH], F32, tag="ot")
                        nc.scalar.copy(ot[:], pso[:, 0:LH])
                        nc.sync.dma_start(o_scan[m * 128:(m + 1) * 128, off_h:off_h + LH], ot[:])

    split_sync_waits(nc)
    return nc


_CACHE = {}


def _get_nc():
    if "nc" not in _CACHE:
        _CACHE["nc"] = build_nc()
    return _CACHE["nc"]


def _prep_in_maps(inputs):
    f16, f32 = np.float16, np.float32
    hidden = np.asarray(inputs["hidden_states"], f32)      # (B, L, DM)
    in_proj_w = np.asarray(inputs["in_proj_w"], f32)       # (5120, 512)
    conv1d_w = np.asarray(inputs["conv1d_w"], f32)         # (DI, 1, 4)
    conv1d_b = np.asarray(inputs["conv1d_b"], f32)
    x_proj_w = np.asarray(inputs["x_proj_w"], f32)         # (64, DI)
    dt_proj_w = np.asarray(inputs["dt_proj_w"], f32)       # (DI, 32)
    dt_proj_b = np.asarray(inputs["dt_proj_b"], f32)
    A = -np.exp(np.asarray(inputs["A_log"], f32))          # (DI, DS)
    D = np.asarray(inputs["D"], f32)
    out_proj_w = np.asarray(inputs["out_proj_w"], f32)     # (512, 3072)
    cb_local_w = np.asarray(inputs["cb_local_w"], f32)     # (512,1,3)
    cb_local_b = np.asarray(inputs["cb_local_b"], f32)
    cb_global_w = np.asarray(inputs["cb_global_w"], f32)   # (512,1,1)
    cb_global_b = np.asarray(inputs["cb_global_b"], f32)
    cb_pre_w = np.asarray(inputs["cb_pre_w"], f32)         # (3,512,1,1)
    cb_pre_b = np.asarray(inputs["cb_pre_b"], f32)         # (3,512)
    cb_dil_w = np.asarray(inputs["cb_dil_w"], f32)         # (3,512,1,3)
    cb_dil_b = np.asarray(inputs["cb_dil_b"], f32)
    cb_ln_g = np.asarray(inputs["cb_ln_g"], f32)           # (2048,)
    cb_ln_b = np.asarray(inputs["cb_ln_b"], f32)
    cb_fuse_w = np.asarray(inputs["cb_fuse_w"], f32)       # (1024, 2048, 1)
    cb_fuse_b = np.asarray(inputs["cb_fuse_b"], f32)

    # host precomputes
    M_comb = out_proj_w[:, 2 * DI:] @ cb_fuse_w[:, :, 0]           # (512, 2048)
    cbias_vec = out_proj_w[:, 2 * DI:] @ cb_fuse_b                 # (512,)
    hmean = hidden.mean(axis=1)                                    # (B, 512)
    W_xc = in_proj_w[4 * DI + DM:4 * DI + 2 * DM]                  # (512, 512) -> xc rows
    xcm_mean = hmean @ W_xc.T                                      # (B, 512)
    phi = np.maximum(cb_global_w[:, 0, 0][None, :] * xcm_mean + cb_global_b[None, :], 0.0)

    def lhsT3(w, kdim=128):  # (K, M) -> (128, K//128, M)
        K, M = w.shape
        return np.ascontiguousarray(w.reshape(K // kdim, kdim, M).transpose(1, 0, 2))

    def perpart(v):  # (n*128,) -> (128, n)
        return np.ascontiguousarray(v.reshape(-1, 128).T)

    selT = np.zeros((32, 2 * DS * 128), f16)
    for n in range(DS):
        selT[n, n * 128:(n + 1) * 128] = 1.0
        selT[DS + n, DS * 128 + n * 128:DS * 128 + (n + 1) * 128] = 1.0

    common = dict(
        cw=np.ascontiguousarray(conv1d_w[:, 0, :].reshape(8, 128, 4).transpose(1, 0, 2).reshape(128, 32)),
        cbias=perpart(conv1d_b),
        xpT=lhsT3(x_proj_w.T).astype(f16),
        selT=selT,
        dpT=np.ascontiguousarray(dt_proj_w.T).astype(f16),
        dpb=perpart(dt_proj_b),
        Asb=np.ascontiguousarray(A.reshape(8, 128, DS).transpose(1, 0, 2).reshape(128, 128)),
        Dsb=perpart(D),
        ident=np.eye(128, dtype=f16),
        prew=np.ascontiguousarray(cb_pre_w[:, :, 0, 0].reshape(3, 4, 128).transpose(2, 0, 1).reshape(128, 12)),
        preb=np.ascontiguousarray(cb_pre_b.reshape(3, 4, 128).transpose(2, 0, 1).reshape(128, 12)),
        dilw=np.ascontiguousarray(cb_dil_w[:, :, 0, :].reshape(3, 4, 128, 3).transpose(2, 0, 1, 3).reshape(128, 36)),
        dilb=np.ascontiguousarray(cb_dil_b.reshape(3, 4, 128).transpose(2, 0, 1).reshape(128, 12)),
        locw=np.ascontiguousarray(cb_local_w[:, 0, :].reshape(4, 128, 3).transpose(1, 0, 2).reshape(128, 12)),
        locb=perpart(cb_local_b),
        lng=perpart(cb_ln_g),
        lnb=perpart(cb_ln_b),
        mcombT=lhsT3(M_comb.T).astype(f16),
        wxz1T=lhsT3(in_proj_w[4 * DI:].T).astype(f16),
    )
    common = {k: np.ascontiguousarray(v) for k, v in common.items()}

    in_maps = []
    for c in range(NC8):
        b, dirn = c % 4, c // 4
        bc, halfc = c // 2, c % 2
        hT_b = hidden[b].T                                  # (512, L)
        if dirn == 1:
            hT_b = hT_b[:, ::-1]
        W1 = in_proj_w[dirn * 2 * DI:(dirn + 1) * 2 * DI]   # (2048, 512)
        Wop = out_proj_w[:, dirn * DI:(dirn + 1) * DI]      # (512, 1024)
        # conv window [start-4, end+4) zero-padded outside [0, L)
        s0 = halfc * LH - 4
        win = np.zeros((DM, CEXT), f32)
        mask = np.zeros((1, CEXT), f16)
        lo, hi = max(s0, 0), min(s0 + CEXT, L)
        win[:, lo - s0:hi - s0] = hidden[bc].T[:, lo:hi]
        mask[0, lo - s0:hi - s0] = 1.0
        in_maps.append(dict(
            common,
            hT=hT_b.astype(f16),
            hTc=win.astype(f16),
            mask=mask,
            w1T=lhsT3(W1.T).astype(f16),
            wopT=lhsT3(Wop.T).astype(f16),
            phi_i=perpart(phi[bc]),
        ))
    in_maps = [{k: np.ascontiguousarray(v) for k, v in m.items()} for m in in_maps]
    return in_maps, cbias_vec


def _assemble(results, cbias_vec):
    out = np.zeros((B, L, DM), np.float32)
    for c in range(NC8):
        b, dirn = c % 4, c // 4
        bc, halfc = c // 2, c % 2
        oscan = results[c]["o_scan"]          # (512, L)
        if dirn == 1:
            oscan = oscan[:, ::-1]
        out[b] += oscan.T
        out[bc, halfc * LH:(halfc + 1) * LH] += results[c]["o_conv"].T
    out += cbias_vec[None, None, :]
    return out


def kernel(**inputs):
    nc = _get_nc()
    in_maps, cbias_vec = _prep_in_maps(inputs)
    res = run_bass_kernel_spmd(nc, in_maps, list(range(NC8)))
    return _assemble(res.results, cbias_vec)


if __name__ == "__main__":
    rng = np.random.default_rng(0)
    dummy = {
        "hidden_states": rng.normal(size=(B, L, DM)).astype(np.float32),
        "in_proj_w": rng.normal(size=(5 * DI, DM)).astype(np.float32) * 0.02,
        "conv1d_w": rng.normal(size=(DI, 1, DC)).astype(np.float32) * 0.2,
        "conv1d_b": np.zeros(DI, np.float32),
        "x_proj_w": rng.normal(size=(DR + 2 * DS, DI)).astype(np.float32) * 0.02,
        "dt_proj_w": rng.uniform(-DR ** -0.5, DR ** -0.5, size=(DI, DR)).astype(np.float32),
        "dt_proj_b": rng.uniform(-5, -1, size=DI).astype(np.float32),
        "A_log": np.log(np.broadcast_to(np.arange(1, DS + 1, dtype=np.float32), (DI, DS))),
        "D": np.ones(DI, np.float32),
        "out_proj_w": rng.normal(size=(DM, 3 * DI)).astype(np.float32) * 0.02,
        "cb_local_w": rng.normal(size=(DM, 1, 3)).astype(np.float32) * 0.2,
        "cb_local_b": np.zeros(DM, np.float32),
        "cb_global_w": rng.normal(size=(DM, 1, 1)).astype(np.float32) * 0.2,
        "cb_global_b": np.zeros(DM, np.float32),
        "cb_pre_w": rng.normal(size=(3, DM, 1, 1)).astype(np.float32) * 0.2,
        "cb_pre_b": np.zeros((3, DM), np.float32),
        "cb_dil_w": rng.normal(size=(3, DM, 1, 3)).astype(np.float32) * 0.2,
        "cb_dil_b": np.zeros((3, DM), np.float32),
        "cb_ln_g": np.ones(4 * DM, np.float32),
        "cb_ln_b": np.zeros(4 * DM, np.float32),
        "cb_fuse_w": rng.normal(size=(2 * DM, 4 * DM, 1)).astype(np.float32) * 0.02,
        "cb_fuse_b": np.zeros(2 * DM, np.float32),
    }
    out = kernel(**dummy)
    print("kernel ran, out shape", out.shape, "finite:", np.isfinite(out).all())



# revision 13
# speedup vs baseline: 2689.0759x; 2689.0759x over previous
"""Trainium2 Bass kernel for nn_ConvmambaProj (bidirectional mamba + dilated-conv branch).

Sharding: 8 cores = (batch b, direction dir) for the mamba scan path, plus
(batch bc, L-half) for the conv branch. Zero cross-core communication; host
does flips/transposes/partial-sum assembly.

v2: engine-balanced. Selective-scan recurrences run on GPSIMD (Pool), freeing
the vector engine for the dBu/hC elementwise products. All depthwise convs
(causal k=4, dilated k=3, local k=3) run on the tensor engine as diagonal
matmuls accumulated in PSUM. The conv-branch LayerNorm is folded into the
fused output matmul (rank-1 mean correction + post-multiply by 1/sd), and the
mask handling is reduced to a per-partition bias plus tiny edge fixups.
"""
import sys

sys.path.insert(0, "/opt/trn_rl_repo")
import numpy as np
import concourse.bass as bass
import concourse.mybir as mybir
from concourse import tile
from concourse.bass_utils import run_bass_kernel_spmd

dt = mybir.dt
AF = mybir.ActivationFunctionType
ALU = mybir.AluOpType

B, L, DM, DI, DS, DR, DC = 4, 2304, 512, 1024, 16, 32, 4
LH = L // 2          # 1152, scan half
NC8 = 8
CEXT = LH + 8        # conv-branch window width (halo 4 each side)
F32, F16 = dt.float32, dt.float16

# ---- tuning knobs ----
SCAN_ON_POOL = False      # walrus rejects Pool tensor_tensor_scan
# GPSIMD elementwise is 4.6x slower than DVE AND contends with DVE's shared
# SBUF ports (exclusive lock) -- measured: DVE scans slow 3-6x while Pool
# runs. Keep Pool to memsets only.
POOL_N = set()
# dA states produced by ACT exp vs derived by DVE squaring (dA_{2k} = dA_k^2).
# pairs are (target_n, source_n) applied in order; remaining n's use ACT exp.
SQ_PAIRS = []
_SQ_TARGETS = {t for t, _ in SQ_PAIRS}


def _nchunks(total, step=512):
    out = []
    o = 0
    while o < total:
        out.append((o, min(step, total - o)))
        o += step
    return out


def split_sync_waits(nc, max_waits=1):
    for f in nc.m.functions:
        for blk in f.blocks:
            new_insts = []
            for inst in blk.instructions:
                si = getattr(inst, "sync_info", None)
                if si and si.on_wait and len(si.on_wait) > max_waits:
                    extra, keep = si.on_wait[:-max_waits], si.on_wait[-max_waits:]
                    for w in extra:
                        new_insts.append(
                            mybir.InstNoOp(
                                name=nc.get_next_instruction_name(),
                                ins=[],
                                outs=[],
                                sync_info=mybir.SyncInfo(on_wait=[w], on_update=[]),
                                engine=inst.engine,
                            )
                        )
                    inst.sync_info = mybir.SyncInfo(on_wait=keep, on_update=si.on_update)
                new_insts.append(inst)
            blk.instructions = new_insts


def build_nc():
    nc = bass.Bass()

    # ---- external inputs (per core) ----
    hT = nc.dram_tensor("hT", [DM, L], F16, kind="ExternalInput")          # hidden[b].T (flipped if bwd)
    hTc = nc.dram_tensor("hTc", [DM, CEXT], F16, kind="ExternalInput")     # conv window of hidden[bc].T
    w1T = nc.dram_tensor("w1T", [128, 4, 2 * DI], F16, kind="ExternalInput")
    wxz1T = nc.dram_tensor("wxz1T", [128, 4, DI], F16, kind="ExternalInput")
    cdiag = nc.dram_tensor("cdiag", [128, 32, 128], F16, kind="ExternalInput")
    cbias = nc.dram_tensor("cbias", [128, 8], F32, kind="ExternalInput")
    xpT = nc.dram_tensor("xpT", [128, 8, 64], F16, kind="ExternalInput")
    selT = nc.dram_tensor("selT", [32, 2 * DS * 128], F16, kind="ExternalInput")
    dpT = nc.dram_tensor("dpT", [DR, DI], F16, kind="ExternalInput")
    dpb = nc.dram_tensor("dpb", [128, 8], F32, kind="ExternalInput")
    Asb = nc.dram_tensor("Asb", [128, 128], F32, kind="ExternalInput")
    Dsb = nc.dram_tensor("Dsb", [128, 8], F32, kind="ExternalInput")
    wopT = nc.dram_tensor("wopT", [128, 8, DM], F16, kind="ExternalInput")
    ident = nc.dram_tensor("ident", [128, 128], F16, kind="ExternalInput")
    phi_i = nc.dram_tensor("phi_i", [128, 4], F32, kind="ExternalInput")
    dildiag = nc.dram_tensor("dildiag", [128, 36, 128], F16, kind="ExternalInput")
    locdiag = nc.dram_tensor("locdiag", [128, 12, 128], F16, kind="ExternalInput")
    dbias = nc.dram_tensor("dbias", [128, 12], F32, kind="ExternalInput")
    locb = nc.dram_tensor("locb", [128, 4], F32, kind="ExternalInput")
    efixL = nc.dram_tensor("efixL", [128, 48], F16, kind="ExternalInput")
    efixR = nc.dram_tensor("efixR", [128, 48], F16, kind="ExternalInput")
    mcombT = nc.dram_tensor("mcombT", [128, 16, DM], F16, kind="ExternalInput")  # Mg^T (ln_g folded)
    mgsumT = nc.dram_tensor("mgsumT", [1, DM], F16, kind="ExternalInput")        # -rowsum(Mg)

    # ---- outputs ----
    o_scan = nc.dram_tensor("o_scan", [DM, L], F32, kind="ExternalOutput")
    o_conv = nc.dram_tensor("o_conv", [DM, LH], F32, kind="ExternalOutput")

    # ---- internal DRAM scratch ----
    zbuf = nc.dram_tensor("zbuf", [8, 128, L], F16)
    xbuf = nc.dram_tensor("xbuf", [8, 128, L], F16)
    dbuf = nc.dram_tensor("dbuf", [8, 128, L], F16)   # delta
    ubuf = nc.dram_tensor("ubuf", [8, 128, L], F16)   # du = delta*x

    with tile.TileContext(nc) as tc:
        with tc.tile_pool(name="pc", bufs=1) as pc:
            # persistent small weights
            cb_t = pc.tile([128, 8], F32, tag="cb", name="cb_t")
            nc.sync.dma_start(cb_t[:], cbias[:])
            xpT_t = pc.tile([128, 8, 64], F16, tag="xpT", name="xpT_t")
            nc.sync.dma_start(xpT_t[:], xpT[:])
            selT_t = pc.tile([32, 2 * DS * 128], F16, tag="selT", name="selT_t")
            nc.sync.dma_start(selT_t[:], selT[:])
            dpT_t = pc.tile([DR, DI], F16, tag="dpT", name="dpT_t")
            nc.sync.dma_start(dpT_t[:], dpT[:])
            dpb_t = pc.tile([128, 8], F32, tag="dpb", name="dpb_t")
            nc.sync.dma_start(dpb_t[:], dpb[:])
            Asb_t = pc.tile([128, 128], F32, tag="Asb", name="Asb_t")
            nc.sync.dma_start(Asb_t[:], Asb[:])
            Dsb_t = pc.tile([128, 8], F32, tag="Dsb", name="Dsb_t")
            nc.sync.dma_start(Dsb_t[:], Dsb[:])
            wopT_t = pc.tile([128, 8, DM], F16, tag="wopT", name="wopT_t")
            nc.sync.dma_start(wopT_t[:], wopT[:])
            id_t = pc.tile([128, 128], F16, tag="ident", name="id_t")
            nc.sync.dma_start(id_t[:], ident[:])
            xdbl_sb = pc.tile([64, L], F16, tag="xdbl", name="xdbl_sb")
            xbc_sb = pc.tile([2 * DS, L], F16, tag="xbc", name="xbc_sb")

            # ============ Phase A: in_proj + conv1d(PE) + silu ============
            with (
                tc.tile_pool(name="pa", bufs=1) as pa,
                tc.tile_pool(name="pxp", bufs=2) as pxp,
                tc.tile_pool(name="px", bufs=8) as px,
                tc.tile_pool(name="pzt", bufs=2) as pzt,
                tc.tile_pool(name="pps", bufs=1, space="PSUM") as pps,
                tc.tile_pool(name="pcv", bufs=1, space="PSUM") as pcv,
            ):
                hT_t = pa.tile([128, 4, L], F16, tag="hT", name="hT_t")
                for k in range(4):
                    nc.sync.dma_start(hT_t[:, k, :], hT[k * 128:(k + 1) * 128, :])
                w1T_t = pa.tile([128, 4, 2 * DI], F16, tag="w1T", name="w1T_t")
                nc.sync.dma_start(w1T_t[:], w1T[:])
                cdiag_t = pa.tile([128, 32, 128], F16, tag="cdiag", name="cdiag_t")
                nc.sync.dma_start(cdiag_t[:], cdiag[:])

                x_tiles = []
                for m in range(16):  # 0-7: x channels, 8-15: z channels
                    if m < 8:
                        xp_t = pxp.tile([128, 3 + L], F16, tag="xpre", name="xp_t")
                        nc.gpsimd.memset(xp_t[:, 0:3], 0.0)
                    for half in range(2):
                        ps = pps.tile([128, LH], F32, tag="mm", name="ps_a")
                        for (off, n) in _nchunks(LH):
                            for k in range(4):
                                nc.tensor.matmul(
                                    ps[:, off:off + n],
                                    w1T_t[:, k, m * 128:(m + 1) * 128],
                                    hT_t[:, k, half * LH + off:half * LH + off + n],
                                    start=(k == 0), stop=(k == 3),
                                )
                        if m < 8:
                            nc.scalar.copy(xp_t[:, 3 + half * LH:3 + (half + 1) * LH], ps[:])
                        else:
                            z_t = pzt.tile([128, LH], F16, tag="zt", name="z_t")
                            nc.scalar.activation(z_t[:], ps[:], AF.Silu)
                            nc.sync.dma_start(zbuf[m - 8, :, half * LH:(half + 1) * LH], z_t[:])
                    if m < 8:
                        # causal depthwise conv (k=4) on PE via diag matmuls
                        x_t = px.tile([128, L], F16, tag="x", name="x_t")
                        for half in range(2):
                            cv = pcv.tile([128, LH], F32, tag="cv", name="cv_a")
                            for (off, n) in _nchunks(LH):
                                for j in range(4):
                                    nc.tensor.matmul(
                                        cv[:, off:off + n],
                                        cdiag_t[:, m * 4 + j, :],
                                        xp_t[:, half * LH + off + j:half * LH + off + j + n],
                                        start=(j == 0), stop=(j == 3),
                                    )
                            nc.scalar.activation(x_t[:, half * LH:(half + 1) * LH], cv[:],
                                                 AF.Silu, bias=cb_t[:, m:m + 1])
                        nc.sync.dma_start(xbuf[m, :, :], x_t[:])
                        x_tiles.append(x_t)

                # ============ Phase B: x_proj, delta, du ============
                for half in range(2):
                    ps = pps.tile([64, LH], F32, tag="mm", name="ps_b")
                    for (off, n) in _nchunks(LH):
                        for k in range(8):
                            nc.tensor.matmul(
                                ps[:, off:off + n],
                                xpT_t[:, k, :],
                                x_tiles[k][:, half * LH + off:half * LH + off + n],
                                start=(k == 0), stop=(k == 7),
                            )
                    nc.scalar.copy(xdbl_sb[:, half * LH:(half + 1) * LH], ps[0:64, :])
                # B/C rows to a partition-0-based tile (compute engines cannot
                # shift partitions; DMA can)
                nc.sync.dma_start(xbc_sb[:], xdbl_sb[32:64, :])

                for c in range(8):
                    dl_t = pzt.tile([128, L], F16, tag="dl", name="dl_t")
                    for half in range(2):
                        ps = pps.tile([128, LH], F32, tag="mm", name="ps_d")
                        for (off, n) in _nchunks(LH):
                            nc.tensor.matmul(
                                ps[:, off:off + n],
                                dpT_t[:, c * 128:(c + 1) * 128],
                                xdbl_sb[0:DR, half * LH + off:half * LH + off + n],
                                start=True, stop=True,
                            )
                        # softplus(x) = ln(exp(x) + 1): Softplus has no ACT table here
                        et = pzt.tile([128, LH], F32, tag="et", name="et_t")
                        nc.scalar.activation(et[:], ps[:], AF.Exp, bias=dpb_t[:, c:c + 1])
                        nc.scalar.activation(dl_t[:, half * LH:(half + 1) * LH], et[:],
                                             AF.Ln, bias=1.0)
                    nc.sync.dma_start(dbuf[c, :, :], dl_t[:])
                    du_t = pzt.tile([128, L], F16, tag="du", name="du_t")
                    nc.vector.tensor_mul(du_t[:], dl_t[:], x_tiles[c][:])
                    nc.sync.dma_start(ubuf[c, :, :], du_t[:])

            # ============ Phase D: conv branch ============
            with (
                tc.tile_pool(name="pd1", bufs=1) as pd1,
                tc.tile_pool(name="pd8", bufs=8) as pd8,
                tc.tile_pool(name="pd16", bufs=16) as pd16,
                tc.tile_pool(name="pdt", bufs=2) as pdt,
                tc.tile_pool(name="pdm", bufs=1, space="PSUM") as pdm,
                tc.tile_pool(name="pdc", bufs=1, space="PSUM") as pdc,
            ):
                hTc_t = pd1.tile([128, 4, CEXT], F16, tag="hTc", name="hTc_t")
                for k in range(4):
                    nc.sync.dma_start(hTc_t[:, k, :], hTc[k * 128:(k + 1) * 128, :])
                wxz1T_t = pd1.tile([128, 4, DI], F16, tag="wxz1T", name="wxz1T_t")
                nc.sync.dma_start(wxz1T_t[:], wxz1T[:])
                mcombT_t = pd1.tile([128, 16, DM], F16, tag="mcombT", name="mcombT_t")
                nc.sync.dma_start(mcombT_t[:], mcombT[:])
                mgsumT_t = pd1.tile([1, DM], F16, tag="mgsumT", name="mgsumT_t")
                nc.sync.dma_start(mgsumT_t[:], mgsumT[:])
                dildiag_t = pd1.tile([128, 36, 128], F16, tag="dildiag", name="dildiag_t")
                nc.sync.dma_start(dildiag_t[:], dildiag[:])
                locdiag_t = pd1.tile([128, 12, 128], F16, tag="locdiag", name="locdiag_t")
                nc.sync.dma_start(locdiag_t[:], locdiag[:])
                dbias_t = pd1.tile([128, 12], F32, tag="dbias", name="dbias_t")
                nc.sync.dma_start(dbias_t[:], dbias[:])
                locb_t = pd1.tile([128, 4], F32, tag="locb", name="locb_t")
                nc.sync.dma_start(locb_t[:], locb[:])
                efixL_t = pd1.tile([128, 48], F16, tag="efixL", name="efixL_t")
                nc.sync.dma_start(efixL_t[:], efixL[:])
                efixR_t = pd1.tile([128, 48], F16, tag="efixR", name="efixR_t")
                nc.sync.dma_start(efixR_t[:], efixR[:])
                phi_t = pd1.tile([128, 4], F32, tag="phi", name="phi_t")
                nc.sync.dma_start(phi_t[:], phi_i[:])
                ones_t = pd1.tile([128, 1], F16, tag="ones", name="ones_t")
                nc.gpsimd.memset(ones_t[:], 1.0)
                one1f = pd1.tile([1, 128], F16, tag="one1f", name="one1f_t")
                nc.gpsimd.memset(one1f[:], 1.0)

                # xz1 = in_proj[4096:5120] @ hidden_window ; m 0-3: xa, 4-7: xc
                xz_tiles = []
                for m in range(8):
                    ps = pdm.tile([128, CEXT], F32, tag="dmm", name="ps_xz")
                    for (off, n) in _nchunks(CEXT):
                        for k in range(4):
                            nc.tensor.matmul(
                                ps[:, off:off + n],
                                wxz1T_t[:, k, m * 128:(m + 1) * 128],
                                hTc_t[:, k, off:off + n],
                                start=(k == 0), stop=(k == 3),
                            )
                    t = pd8.tile([128, CEXT], F16, tag="xz", name="xz_t")
                    nc.scalar.copy(t[:], ps[:])
                    xz_tiles.append(t)
                xa_tiles, xc_tiles = xz_tiles[:4], xz_tiles[4:]

                cat_tiles = []
                # feats: 3 dilations x 4 ch-tiles (cat channels 0..1535), PE conv
                for i, d in enumerate((1, 2, 4)):
                    for t4 in range(4):
                        cv = pdc.tile([128, LH], F32, tag="dcv", name="cv_dil")
                        for (off, n) in _nchunks(LH):
                            for j in range(3):
                                nc.tensor.matmul(
                                    cv[:, off:off + n],
                                    dildiag_t[:, (i * 4 + t4) * 3 + j, :],
                                    xa_tiles[t4][:, 4 + (j - 1) * d + off:4 + (j - 1) * d + off + n],
                                    start=(j == 0), stop=(j == 2),
                                )
                        ct = pd16.tile([128, LH], F16, tag="cat", name="ct_dil")
                        nc.scalar.activation(ct[:], cv[:], AF.Identity,
                                             bias=dbias_t[:, i * 4 + t4:i * 4 + t4 + 1])
                        # boundary taps beyond the sequence: host-computed fixups
                        e = (i * 4 + t4) * 4
                        nc.vector.tensor_add(ct[:, 0:4], ct[:, 0:4], efixL_t[:, e:e + 4])
                        nc.vector.tensor_add(ct[:, LH - 4:LH], ct[:, LH - 4:LH],
                                             efixR_t[:, e:e + 4])
                        cat_tiles.append(ct)
                # phi * gelu(local conv + b)  (cat channels 1536..2047), PE conv
                for t4 in range(4):
                    cv = pdc.tile([128, LH], F32, tag="dcv", name="cv_loc")
                    for (off, n) in _nchunks(LH):
                        for j in range(3):
                            nc.tensor.matmul(
                                cv[:, off:off + n],
                                locdiag_t[:, t4 * 3 + j, :],
                                xc_tiles[t4][:, 3 + j + off:3 + j + off + n],
                                start=(j == 0), stop=(j == 2),
                            )
                    lg = pdt.tile([128, LH], F16, tag="lg", name="lg_t")
                    nc.scalar.activation(lg[:], cv[:], AF.Gelu, bias=locb_t[:, t4:t4 + 1])
                    ct = pd16.tile([128, LH], F16, tag="cat", name="ct_loc")
                    nc.vector.tensor_scalar(ct[:], lg[:], phi_t[:, t4:t4 + 1], None, ALU.mult)
                    cat_tiles.append(ct)

                # LN folded into fuse matmul: need mu, rsd = 1/sqrt(var+eps)
                pstat = pdm.tile([1, LH], F32, tag="dmm", name="pstat")
                for (off, n) in _nchunks(LH):
                    for t16 in range(16):
                        nc.tensor.matmul(pstat[0:1, off:off + n], ones_t[:],
                                         cat_tiles[t16][:, off:off + n],
                                         start=(t16 == 0), stop=(t16 == 15),
                                         skip_group_check=True)
                mu = pd1.tile([1, LH], F16, tag="mu", name="mu_t")
                nc.scalar.activation(mu[:], pstat[0:1, :], AF.Copy, scale=1.0 / 2048)
                pstat2 = pdm.tile([1, LH], F32, tag="dmm", name="pstat2")
                for t16 in range(16):
                    sq = pdt.tile([128, LH], F16, tag="sq", name="sq_t")
                    nc.vector.tensor_mul(sq[:], cat_tiles[t16][:], cat_tiles[t16][:])
                    for (off, n) in _nchunks(LH):
                        nc.tensor.matmul(pstat2[0:1, off:off + n], ones_t[:],
                                         sq[:, off:off + n],
                                         start=(t16 == 0), stop=(t16 == 15),
                                         skip_group_check=True)
                ex2 = pd1.tile([1, LH], F32, tag="ex2", name="ex2_t")
                nc.scalar.activation(ex2[:], pstat2[0:1, :], AF.Copy, scale=1.0 / 2048)
                var = pd1.tile([1, LH], F32, tag="var", name="var_t")
                nc.vector.tensor_mul(var[:], mu[:], mu[:])
                nc.vector.tensor_sub(var[:], ex2[:], var[:])
                nc.vector.tensor_scalar_add(var[:], var[:], 1e-5)
                sd = pd1.tile([1, LH], F32, tag="sd", name="sd_t")
                nc.scalar.activation(sd[:], var[:], AF.Sqrt)
                rsd = pd1.tile([1, LH], F16, tag="rsd", name="rsd_t")
                with nc.allow_low_precision(reason="1/sd fits f16; tol 2e-2"):
                    nc.vector.reciprocal(rsd[:], sd[:])
                # replicate rsd to 128 partitions
                rsrep = pd1.tile([128, LH], F16, tag="rsrep", name="rsrep_t")
                psr = pdm.tile([128, LH], F32, tag="dmm", name="ps_rs")
                for (off, n) in _nchunks(LH):
                    nc.tensor.matmul(psr[:, off:off + n], one1f[:], rsd[:, off:off + n],
                                     start=True, stop=True)
                nc.scalar.copy(rsrep[:], psr[:])

                # fused (out_proj[:,2048:] @ cb_fuse_w @ diag(ln_g)) @ cat
                # with rank-1 mean correction, then * (1/sd) broadcast
                for m in range(4):
                    psf = pdc.tile([128, LH], F32, tag="dcv", name="psf_t")
                    for (off, n) in _nchunks(LH):
                        for k in range(16):
                            nc.tensor.matmul(
                                psf[:, off:off + n],
                                mcombT_t[:, k, m * 128:(m + 1) * 128],
                                cat_tiles[k][:, off:off + n],
                                start=(k == 0), stop=False,
                            )
                        nc.tensor.matmul(
                            psf[:, off:off + n],
                            mgsumT_t[0:1, m * 128:(m + 1) * 128],
                            mu[0:1, off:off + n],
                            start=False, stop=True,
                        )
                    psf_sb = pdt.tile([128, LH], F16, tag="psfsb", name="psf_sb")
                    nc.scalar.copy(psf_sb[:], psf[:])
                    oc = pdt.tile([128, LH], F32, tag="oc", name="oc_t")
                    nc.vector.tensor_mul(oc[:], psf_sb[:], rsrep[:])
                    nc.sync.dma_start(o_conv[m * 128:(m + 1) * 128, :], oc[:])

            # ============ Phase C: selective scan ============
            with (
                tc.tile_pool(name="pb16", bufs=16) as pb16,
                tc.tile_pool(name="ph1", bufs=1) as ph1,
                tc.tile_pool(name="ps2p", bufs=2) as ps2p,
                tc.tile_pool(name="ps3p", bufs=2) as ps3p,
                tc.tile_pool(name="phl", bufs=8) as phl,
                tc.tile_pool(name="ppy", bufs=2, space="PSUM") as ppy,
            ):
                hlast = [phl.tile([128, DS], F32, tag="hlast", name=f"hlast{i}")
                         for i in range(8)]
                yg_t = ph1.tile([128, 8, LH], F16, tag="yg", name="yg_t")
                for half in range(2):
                    off_h = half * LH
                    # build replicated B/C rows for all 16 states
                    breps, creps = [], []
                    for n in range(DS):
                        for is_c in range(2):
                            psr2 = ppy.tile([128, LH], F32, tag="py", name="ps_rep")
                            for (off, nn) in _nchunks(LH):
                                nc.tensor.matmul(
                                    psr2[:, off:off + nn],
                                    selT_t[:, is_c * DS * 128 + n * 128:(is_c * DS + n + 1) * 128],
                                    xbc_sb[:, off_h + off:off_h + off + nn],
                                    start=True, stop=True,
                                )
                            rt = pb16.tile([128, LH], F16,
                                           tag=("crep" if is_c else "brep"), name="rep_t")
                            nc.scalar.copy(rt[:], psr2[:])
                            (creps if is_c else breps).append(rt)

                    for c in range(8):
                        dl_t = ps2p.tile([128, LH], F16, tag="dls", name="dl_s")
                        nc.sync.dma_start(dl_t[:], dbuf[c, :, off_h:off_h + LH])
                        du_t = ps2p.tile([128, LH], F16, tag="dus", name="du_s")
                        nc.sync.dma_start(du_t[:], ubuf[c, :, off_h:off_h + LH])
                        x_t = ps2p.tile([128, LH], F16, tag="xs", name="x_s")
                        nc.sync.dma_start(x_t[:], xbuf[c, :, off_h:off_h + LH])
                        sz_t = ps2p.tile([128, LH], F16, tag="szs", name="sz_s")
                        nc.sync.dma_start(sz_t[:], zbuf[c, :, off_h:off_h + LH])

                        hb = ph1.tile([128, DS, LH], F16, tag="hb", name="hb_t")
                        dAs = {}
                        psy = ppy.tile([128, LH], F32, tag="py", name="psy_t")
                        for n in range(DS):
                            dA = ps3p.tile([128, LH], F16, tag="dA", name=f"dA_{n % 3}")
                            if n in _SQ_TARGETS:
                                src = dAs[[s for t, s in SQ_PAIRS if t == n][0]]
                                nc.vector.tensor_mul(dA[:], src[:], src[:])
                            else:
                                nc.scalar.activation(dA[:], dl_t[:], AF.Exp,
                                                     scale=Asb_t[:, c * DS + n:c * DS + n + 1])
                            if any(s == n for _, s in SQ_PAIRS):
                                dAs[n] = dA
                            dBu = ps3p.tile([128, LH], F16, tag="dBu", name="dBu_t")
                            nc.vector.tensor_mul(dBu[:], du_t[:], breps[n][:])
                            init = 0.0 if half == 0 else hlast[c][:, n:n + 1]
                            scan_eng = nc.gpsimd if SCAN_ON_POOL else nc.vector
                            scan_eng.tensor_tensor_scan(hb[:, n, :], dA[:], dBu[:], init,
                                                        ALU.mult, ALU.add)
                            hC = ps3p.tile([128, LH], F16, tag="hC", name="hC_t")
                            nc.vector.tensor_mul(hC[:], hb[:, n, :], creps[n][:])
                            for (off, nn) in _nchunks(LH):
                                nc.tensor.matmul(psy[:, off:off + nn], id_t[:], hC[:, off:off + nn],
                                                 start=(n == 0), stop=(n == DS - 1),
                                                 skip_group_check=True)
                        if half == 0:
                            nc.vector.tensor_copy(hlast[c][:, :], hb[:, :, LH - 1])
                        # epilogue: yg = (y + D*x) * silu(z); psy -> SBUF via ACT
                        # so the DVE ops run in 2x mode
                        psy_sb = ps2p.tile([128, LH], F16, tag="psysb", name="psy_sb")
                        nc.scalar.copy(psy_sb[:], psy[:])
                        tmp = ps2p.tile([128, LH], F16, tag="tmp", name="tmp_t")
                        nc.vector.scalar_tensor_tensor(tmp[:], x_t[:], Dsb_t[:, c:c + 1],
                                                       psy_sb[:], ALU.mult, ALU.add)
                        nc.vector.tensor_mul(yg_t[:, c, :], tmp[:], sz_t[:])

                    # out_proj partial for this half
                    for m in range(4):
                        pso = ppy.tile([128, LH], F32, tag="py", name="pso_t")
                        for (off, nn) in _nchunks(LH):
                            for c in range(8):
                                nc.tensor.matmul(
                                    pso[:, off:off + nn],
                                    wopT_t[:, c, m * 128:(m + 1) * 128],
                                    yg_t[:, c, off:off + nn],
                                    start=(c == 0), stop=(c == 7),
                                )
                        ot = ps2p.tile([128, LH], F32, tag="ot", name="ot_t")
                        nc.scalar.copy(ot[:], pso[:])
                        nc.sync.dma_start(o_scan[m * 128:(m + 1) * 128, off_h:off_h + LH], ot[:])

    split_sync_waits(nc)
    return nc


_CACHE = {}


def _get_nc():
    if "nc" not in _CACHE:
        _CACHE["nc"] = build_nc()
    return _CACHE["nc"]


def _prep_in_maps(inputs):
    f16, f32 = np.float16, np.float32
    hidden = np.asarray(inputs["hidden_states"], f32)      # (B, L, DM)
    in_proj_w = np.asarray(inputs["in_proj_w"], f32)       # (5120, 512)
    conv1d_w = np.asarray(inputs["conv1d_w"], f32)         # (DI, 1, 4)
    conv1d_b = np.asarray(inputs["conv1d_b"], f32)
    x_proj_w = np.asarray(inputs["x_proj_w"], f32)         # (64, DI)
    dt_proj_w = np.asarray(inputs["dt_proj_w"], f32)       # (DI, 32)
    dt_proj_b = np.asarray(inputs["dt_proj_b"], f32)
    A = -np.exp(np.asarray(inputs["A_log"], f32))          # (DI, DS)
    D = np.asarray(inputs["D"], f32)
    out_proj_w = np.asarray(inputs["out_proj_w"], f32)     # (512, 3072)
    cb_local_w = np.asarray(inputs["cb_local_w"], f32)     # (512,1,3)
    cb_local_b = np.asarray(inputs["cb_local_b"], f32)
    cb_global_w = np.asarray(inputs["cb_global_w"], f32)   # (512,1,1)
    cb_global_b = np.asarray(inputs["cb_global_b"], f32)
    cb_pre_w = np.asarray(inputs["cb_pre_w"], f32)         # (3,512,1,1)
    cb_pre_b = np.asarray(inputs["cb_pre_b"], f32)         # (3,512)
    cb_dil_w = np.asarray(inputs["cb_dil_w"], f32)         # (3,512,1,3)
    cb_dil_b = np.asarray(inputs["cb_dil_b"], f32)
    cb_ln_g = np.asarray(inputs["cb_ln_g"], f32)           # (2048,)
    cb_ln_b = np.asarray(inputs["cb_ln_b"], f32)
    cb_fuse_w = np.asarray(inputs["cb_fuse_w"], f32)       # (1024, 2048, 1)
    cb_fuse_b = np.asarray(inputs["cb_fuse_b"], f32)

    # host precomputes
    Wop3 = out_proj_w[:, 2 * DI:]                                  # (512, 1024)
    M = Wop3 @ cb_fuse_w[:, :, 0]                                  # (512, 2048)
    Mg = M * cb_ln_g[None, :]                                      # ln gain folded
    cbias_vec = Wop3 @ cb_fuse_b + M @ cb_ln_b                     # (512,)
    mgsum = Mg.sum(axis=1)                                         # (512,)
    hmean = hidden.mean(axis=1)                                    # (B, 512)
    W_xc = in_proj_w[4 * DI + DM:4 * DI + 2 * DM]                  # (512, 512) -> xc rows
    xcm_mean = hmean @ W_xc.T                                      # (B, 512)
    phi = np.maximum(cb_global_w[:, 0, 0][None, :] * xcm_mean + cb_global_b[None, :], 0.0)

    def lhsT3(w, kdim=128):  # (K, M) -> (128, K//128, M)
        K, Mm = w.shape
        return np.ascontiguousarray(w.reshape(K // kdim, kdim, Mm).transpose(1, 0, 2))

    def perpart(v):  # (n*128,) -> (128, n)
        return np.ascontiguousarray(v.reshape(-1, 128).T)

    def diagpack(vals):  # (nmat, 128) -> (128, nmat, 128) diagonal lhsT mats
        nmat = vals.shape[0]
        out = np.zeros((128, nmat, 128), f16)
        idx = np.arange(128)
        for i in range(nmat):
            out[idx, i, idx] = vals[i]
        return out

    selT = np.zeros((32, 2 * DS * 128), f16)
    for n in range(DS):
        selT[n, n * 128:(n + 1) * 128] = 1.0
        selT[DS + n, DS * 128 + n * 128:DS * 128 + (n + 1) * 128] = 1.0

    # conv1d diag: vals[m*4+j, p] = w[m*128+p, j]
    cw2 = conv1d_w[:, 0, :].reshape(8, 128, 4)
    cvals = np.ascontiguousarray(cw2.transpose(0, 2, 1).reshape(32, 128))
    # dilated conv diag: vals[(i*4+t4)*3+j, p] = dil_w[i, t4*128+p, j] * pre_w[i, t4*128+p]
    dw = cb_dil_w[:, :, 0, :]                                      # (3, 512, 3)
    pw = cb_pre_w[:, :, 0, 0]                                      # (3, 512)
    dvals = np.zeros((36, 128), f32)
    for i in range(3):
        for t4 in range(4):
            for j in range(3):
                ch = slice(t4 * 128, (t4 + 1) * 128)
                dvals[(i * 4 + t4) * 3 + j] = dw[i, ch, j] * pw[i, ch]
    # local conv diag: vals[t4*3+j, p] = local_w[t4*128+p, j]
    lvals = np.zeros((12, 128), f32)
    for t4 in range(4):
        for j in range(3):
            lvals[t4 * 3 + j] = cb_local_w[t4 * 128:(t4 + 1) * 128, 0, j]
    # dil conv bias: preb*sum_j(dilw) + dilb, per (i, t4) per partition
    dbias_m = np.zeros((128, 12), f32)
    for i in range(3):
        for t4 in range(4):
            ch = slice(t4 * 128, (t4 + 1) * 128)
            dbias_m[:, i * 4 + t4] = (cb_pre_b[i, ch] * dw[i, ch, :].sum(-1)
                                      + cb_dil_b[i, ch])

    common = dict(
        cbias=perpart(conv1d_b),
        xpT=lhsT3(x_proj_w.T).astype(f16),
        selT=selT,
        dpT=np.ascontiguousarray(dt_proj_w.T).astype(f16),
        dpb=perpart(dt_proj_b),
        Asb=np.ascontiguousarray(A.reshape(8, 128, DS).transpose(1, 0, 2).reshape(128, 128)),
        Dsb=perpart(D),
        ident=np.eye(128, dtype=f16),
        cdiag=diagpack(cvals),
        dildiag=diagpack(dvals),
        locdiag=diagpack(lvals),
        dbias=dbias_m,
        locb=perpart(cb_local_b),
        mcombT=lhsT3(Mg.T).astype(f16),
        mgsumT=np.ascontiguousarray(-mgsum[None, :]).astype(f16),
        wxz1T=lhsT3(in_proj_w[4 * DI:].T).astype(f16),
    )
    common = {k: np.ascontiguousarray(v) for k, v in common.items()}

    in_maps = []
    for c in range(NC8):
        b, dirn = c % 4, c // 4
        bc, halfc = c // 2, c % 2
        hT_b = hidden[b].T                                  # (512, L)
        if dirn == 1:
            hT_b = hT_b[:, ::-1]
        W1 = in_proj_w[dirn * 2 * DI:(dirn + 1) * 2 * DI]   # (2048, 512)
        Wop = out_proj_w[:, dirn * DI:(dirn + 1) * DI]      # (512, 1024)
        # conv window [start-4, end+4) zero-padded outside [0, L)
        s0 = halfc * LH - 4
        win = np.zeros((DM, CEXT), f32)
        lo, hi = max(s0, 0), min(s0 + CEXT, L)
        win[:, lo - s0:hi - s0] = hidden[bc].T[:, lo:hi]
        # edge fixups for the dilated-conv preb term: columns where a tap
        # falls outside [0, L).  halfc=0 -> left edge (t < d, j=0 tap),
        # halfc=1 -> right edge (t >= LH-d, j=2 tap).  value = -preb*dilw_j
        efl = np.zeros((128, 48), f16)
        efr = np.zeros((128, 48), f16)
        for i, dd in enumerate((1, 2, 4)):
            for t4 in range(4):
                ch = slice(t4 * 128, (t4 + 1) * 128)
                e = (i * 4 + t4) * 4
                if halfc == 0:
                    for t in range(min(dd, 4)):
                        efl[:, e + t] = -(cb_pre_b[i, ch] * dw[i, ch, 0])
                else:
                    for tt in range(4):
                        if tt >= 4 - dd:
                            efr[:, e + tt] = -(cb_pre_b[i, ch] * dw[i, ch, 2])
        in_maps.append(dict(
            common,
            hT=hT_b.astype(f16),
            hTc=win.astype(f16),
            w1T=lhsT3(W1.T).astype(f16),
            wopT=lhsT3(Wop.T).astype(f16),
            phi_i=perpart(phi[bc]),
            efixL=efl,
            efixR=efr,
        ))
    in_maps = [{k: np.ascontiguousarray(v) for k, v in m.items()} for m in in_maps]
    return in_maps, cbias_vec


def _assemble(results, cbias_vec):
    out = np.zeros((B, L, DM), np.float32)
    for c in range(NC8):
        b, dirn = c % 4, c // 4
        bc, halfc = c // 2, c % 2
        oscan = results[c]["o_scan"]          # (512, L)
        if dirn == 1:
            oscan = oscan[:, ::-1]
        out[b] += oscan.T
        out[bc, halfc * LH:(halfc + 1) * LH] += results[c]["o_conv"].T
    out += cbias_vec[None, None, :]
    return out


def kernel(**inputs):
    nc = _get_nc()
    in_maps, cbias_vec = _prep_in_maps(inputs)
    res = run_bass_kernel_spmd(nc, in_maps, list(range(NC8)))
    return _assemble(res.results, cbias_vec)


# revision 18
# speedup vs baseline: 3335.6992x; 1.2405x over previous
"""Trainium2 Bass kernel for nn_ConvmambaProj (bidirectional mamba + dilated-conv branch).

Sharding: 8 cores = (batch b, direction dir) for the mamba scan path, plus
(batch bc, L-half) for the conv branch. Zero cross-core communication; host
does flips/transposes/partial-sum assembly.

v3: the vector engine is the hard bottleneck (scan recurrences are DVE-only on
this chip), so everything else is arranged around keeping it busy:
- dBu = delta*u*B products for all (c,n) are precomputed into DRAM during the
  PE/ACT-heavy prologue (in_proj/convs/conv-branch) where the DVE would
  otherwise idle, then streamed back during the scan loop.
- Scans run concatenated 4 states per instruction (state reset via zeroed
  first dA column; half-1 carry-in patched into dBu), amortizing DVE drains.
- hC multiplies are batched 4 states per instruction against a contiguous
  replicated-C tile.
- Per-channel-tile epilogues are deferred by one tile so their PE/ACT/DVE
  chain overlaps the next tile's scans.
- All depthwise convs run on the tensor engine as diagonal matmuls; the
  conv-branch LayerNorm is folded into the fused output matmul.
"""
import sys

sys.path.insert(0, "/opt/trn_rl_repo")
import numpy as np
import concourse.bass as bass
import concourse.mybir as mybir
from concourse import tile
from concourse.bass_utils import run_bass_kernel_spmd

dt = mybir.dt
AF = mybir.ActivationFunctionType
ALU = mybir.AluOpType

B, L, DM, DI, DS, DR, DC = 4, 2304, 512, 1024, 16, 32, 4
LH = L // 2          # 1152, scan half
NC8 = 8
CEXT = LH + 8        # conv-branch window width (halo 4 each side)
F32, F16 = dt.float32, dt.float16
Q = 4                # states concatenated per scan instruction
NQ = DS // Q         # scan groups per (c, half)
QL = Q * LH


def _nchunks(total, step=512):
    out = []
    o = 0
    while o < total:
        out.append((o, min(step, total - o)))
        o += step
    return out


def split_sync_waits(nc, max_waits=1):
    for f in nc.m.functions:
        for blk in f.blocks:
            new_insts = []
            for inst in blk.instructions:
                si = getattr(inst, "sync_info", None)
                if si and si.on_wait and len(si.on_wait) > max_waits:
                    extra, keep = si.on_wait[:-max_waits], si.on_wait[-max_waits:]
                    for w in extra:
                        new_insts.append(
                            mybir.InstNoOp(
                                name=nc.get_next_instruction_name(),
                                ins=[],
                                outs=[],
                                sync_info=mybir.SyncInfo(on_wait=[w], on_update=[]),
                                engine=inst.engine,
                            )
                        )
                    inst.sync_info = mybir.SyncInfo(on_wait=keep, on_update=si.on_update)
                new_insts.append(inst)
            blk.instructions = new_insts


def build_nc():
    nc = bass.Bass()

    # ---- external inputs (per core) ----
    hT = nc.dram_tensor("hT", [DM, L], F16, kind="ExternalInput")          # hidden[b].T (flipped if bwd)
    hTc = nc.dram_tensor("hTc", [DM, CEXT], F16, kind="ExternalInput")     # conv window of hidden[bc].T
    w1T = nc.dram_tensor("w1T", [128, 4, 2 * DI], F16, kind="ExternalInput")
    wxz1T = nc.dram_tensor("wxz1T", [128, 4, DI], F16, kind="ExternalInput")
    cdiag = nc.dram_tensor("cdiag", [128, 32, 128], F16, kind="ExternalInput")
    cbias = nc.dram_tensor("cbias", [128, 8], F32, kind="ExternalInput")
    xpT = nc.dram_tensor("xpT", [128, 8, 64], F16, kind="ExternalInput")
    selT = nc.dram_tensor("selT", [32, 2 * DS * 128], F16, kind="ExternalInput")
    dpT = nc.dram_tensor("dpT", [DR, DI], F16, kind="ExternalInput")
    dpb = nc.dram_tensor("dpb", [128, 8], F32, kind="ExternalInput")
    Asb = nc.dram_tensor("Asb", [128, 128], F32, kind="ExternalInput")
    Dsb = nc.dram_tensor("Dsb", [128, 8], F32, kind="ExternalInput")
    wopT = nc.dram_tensor("wopT", [128, 8, DM], F16, kind="ExternalInput")
    ident = nc.dram_tensor("ident", [128, 128], F16, kind="ExternalInput")
    phi_i = nc.dram_tensor("phi_i", [128, 4], F32, kind="ExternalInput")
    dildiag = nc.dram_tensor("dildiag", [128, 36, 128], F16, kind="ExternalInput")
    locdiag = nc.dram_tensor("locdiag", [128, 12, 128], F16, kind="ExternalInput")
    dbias = nc.dram_tensor("dbias", [128, 12], F32, kind="ExternalInput")
    locb = nc.dram_tensor("locb", [128, 4], F32, kind="ExternalInput")
    efixL = nc.dram_tensor("efixL", [128, 48], F16, kind="ExternalInput")
    efixR = nc.dram_tensor("efixR", [128, 48], F16, kind="ExternalInput")
    mcombT = nc.dram_tensor("mcombT", [128, 16, DM], F16, kind="ExternalInput")  # Mg^T (ln_g folded)
    mgsumT = nc.dram_tensor("mgsumT", [1, DM], F16, kind="ExternalInput")        # -rowsum(Mg)

    # ---- outputs ----
    o_scan = nc.dram_tensor("o_scan", [DM, L], F32, kind="ExternalOutput")
    o_conv = nc.dram_tensor("o_conv", [DM, LH], F32, kind="ExternalOutput")

    # ---- internal DRAM scratch ----
    zbuf = nc.dram_tensor("zbuf", [8, 128, L], F16)
    xbuf = nc.dram_tensor("xbuf", [8, 128, L], F16)
    dbuf = nc.dram_tensor("dbuf", [8, 128, L], F16)        # delta
    dbu2 = nc.dram_tensor("dbu2", [8, DS, 128, L], F16)    # delta*u*B per (c,n)

    with tile.TileContext(nc) as tc:
        with tc.tile_pool(name="pc", bufs=1) as pc:
            # persistent small weights
            cb_t = pc.tile([128, 8], F32, tag="cb", name="cb_t")
            nc.sync.dma_start(cb_t[:], cbias[:])
            xpT_t = pc.tile([128, 8, 64], F16, tag="xpT", name="xpT_t")
            nc.sync.dma_start(xpT_t[:], xpT[:])
            selT_t = pc.tile([32, 2 * DS * 128], F16, tag="selT", name="selT_t")
            nc.sync.dma_start(selT_t[:], selT[:])
            dpb_t = pc.tile([128, 8], F32, tag="dpb", name="dpb_t")
            nc.sync.dma_start(dpb_t[:], dpb[:])
            Asb_t = pc.tile([128, 128], F32, tag="Asb", name="Asb_t")
            nc.sync.dma_start(Asb_t[:], Asb[:])
            Dsb_t = pc.tile([128, 8], F32, tag="Dsb", name="Dsb_t")
            nc.sync.dma_start(Dsb_t[:], Dsb[:])
            wopT_t = pc.tile([128, 8, DM], F16, tag="wopT", name="wopT_t")
            nc.sync.dma_start(wopT_t[:], wopT[:])
            id_t = pc.tile([128, 128], F16, tag="ident", name="id_t")
            nc.sync.dma_start(id_t[:], ident[:])
            xbc_sb = pc.tile([2 * DS, L], F16, tag="xbc", name="xbc_sb")

            with tc.tile_pool(name="pa", bufs=1) as pa:
                hT_t = pa.tile([128, 4, L], F16, tag="hT", name="hT_t")
                for k in range(4):
                    nc.sync.dma_start(hT_t[:, k, :], hT[k * 128:(k + 1) * 128, :])
                w1T_t = pa.tile([128, 4, 2 * DI], F16, tag="w1T", name="w1T_t")
                nc.sync.dma_start(w1T_t[:], w1T[:])
                cdiag_t = pa.tile([128, 32, 128], F16, tag="cdiag", name="cdiag_t")
                nc.sync.dma_start(cdiag_t[:], cdiag[:])

                # ===== Phase A-x: in_proj x-channels + causal conv (PE) + silu
                # ===== Phase B:   x_proj, delta; du tiles stay resident
                # ===== dBu pre-compute for the whole scan into DRAM
                with (
                    tc.tile_pool(name="px", bufs=8) as px,
                    tc.tile_pool(name="pdu", bufs=8) as pdu,
                    tc.tile_pool(name="pxp", bufs=2) as pxp,
                    tc.tile_pool(name="pxd", bufs=1) as pxd,
                    tc.tile_pool(name="pzt", bufs=2) as pzt,
                ):
                  with (
                    tc.tile_pool(name="pps", bufs=4, space="PSUM") as pps,
                    tc.tile_pool(name="pcv", bufs=3, space="PSUM") as pcv,
                  ):
                    xdbl_sb = pxd.tile([64, L], F16, tag="xdbl", name="xdbl_sb")
                    dpT_t = pxd.tile([DR, DI], F16, tag="dpT", name="dpT_t")
                    nc.sync.dma_start(dpT_t[:], dpT[:])

                    x_tiles = []
                    for m in range(8):
                        xp_t = pxp.tile([128, 3 + L], F16, tag="xpre", name="xp_t")
                        nc.gpsimd.memset(xp_t[:, 0:3], 0.0)
                        for half in range(2):
                            for (off, n) in _nchunks(LH):
                                go = half * LH + off
                                ps = pps.tile([128, 512], F32, tag="mm", name="ps_a")
                                for k in range(4):
                                    nc.tensor.matmul(
                                        ps[:, 0:n],
                                        w1T_t[:, k, m * 128:(m + 1) * 128],
                                        hT_t[:, k, go:go + n],
                                        start=(k == 0), stop=(k == 3),
                                    )
                                nc.scalar.copy(xp_t[:, 3 + go:3 + go + n], ps[:, 0:n])
                        x_t = px.tile([128, L], F16, tag="x", name="x_t")
                        for half in range(2):
                            for (off, n) in _nchunks(LH):
                                go = half * LH + off
                                cv = pcv.tile([128, 512], F32, tag="cv", name="cv_a")
                                for j in range(4):
                                    nc.tensor.matmul(
                                        cv[:, 0:n],
                                        cdiag_t[:, m * 4 + j, :],
                                        xp_t[:, go + j:go + j + n],
                                        start=(j == 0), stop=(j == 3),
                                    )
                                nc.scalar.activation(x_t[:, go:go + n], cv[:, 0:n],
                                                     AF.Silu, bias=cb_t[:, m:m + 1])
                        nc.sync.dma_start(xbuf[m, :, :], x_t[:])
                        x_tiles.append(x_t)

                    # x_proj -> xdbl (dt rows 0:32, B/C rows 32:64)
                    for half in range(2):
                        for (off, n) in _nchunks(LH):
                            go = half * LH + off
                            ps = pps.tile([64, 512], F32, tag="mm", name="ps_b")
                            for k in range(8):
                                nc.tensor.matmul(
                                    ps[0:64, 0:n],
                                    xpT_t[:, k, :],
                                    x_tiles[k][:, go:go + n],
                                    start=(k == 0), stop=(k == 7),
                                )
                            nc.scalar.copy(xdbl_sb[:, go:go + n], ps[0:64, 0:n])
                    nc.sync.dma_start(xbc_sb[:], xdbl_sb[32:64, :])

                    # delta (softplus) and du = delta*x
                    du_tiles = []
                    for c in range(8):
                        dl_t = pzt.tile([128, L], F16, tag="dl", name="dl_t")
                        for half in range(2):
                            for (off, n) in _nchunks(LH):
                                go = half * LH + off
                                ps = pps.tile([128, 512], F32, tag="mm", name="ps_d")
                                nc.tensor.matmul(
                                    ps[:, 0:n],
                                    dpT_t[:, c * 128:(c + 1) * 128],
                                    xdbl_sb[0:DR, go:go + n],
                                    start=True, stop=True,
                                )
                                et = pzt.tile([128, 512], F32, tag="et", name="et_t")
                                nc.scalar.activation(et[:, 0:n], ps[:, 0:n], AF.Exp,
                                                     bias=dpb_t[:, c:c + 1])
                                nc.scalar.activation(dl_t[:, go:go + n], et[:, 0:n],
                                                     AF.Ln, bias=1.0)
                        nc.sync.dma_start(dbuf[c, :, :], dl_t[:])
                        du_t = pdu.tile([128, L], F16, tag="du8", name="du_t")
                        nc.vector.tensor_mul(du_t[:], dl_t[:], x_tiles[c][:])
                        du_tiles.append(du_t)

                  # dBu = du * B_n for every (c, n) -> DRAM (fills idle DVE
                  # while the PE/ACT run the z-channels + conv branch)
                  with (
                    tc.tile_pool(name="pbr", bufs=2) as pbr,
                    tc.tile_pool(name="pdb", bufs=3) as pdb,
                    tc.tile_pool(name="pbp", bufs=3, space="PSUM") as pbp,
                  ):
                    for n in range(DS):
                        br = pbr.tile([128, L], F16, tag="br", name="br_t")
                        for (off, nn) in _nchunks(L):
                            psb = pbp.tile([128, 512], F32, tag="bmm", name="ps_br")
                            nc.tensor.matmul(
                                psb[:, 0:nn],
                                selT_t[:, n * 128:(n + 1) * 128],
                                xbc_sb[:, off:off + nn],
                                start=True, stop=True,
                            )
                            if (off // 512) % 2 == 0:
                                nc.scalar.copy(br[:, off:off + nn], psb[:, 0:nn])
                            else:
                                nc.vector.tensor_copy(br[:, off:off + nn], psb[:, 0:nn])
                        for c in range(8):
                            db = pdb.tile([128, L], F16, tag="db", name="db_t")
                            nc.vector.tensor_mul(db[:], du_tiles[c][:], br[:])
                            nc.sync.dma_start(dbu2[c, n, :, :], db[:])

                # ===== Phase A-z: in_proj z-channels + silu -> zbuf
                with (
                    tc.tile_pool(name="pzz", bufs=3) as pzz,
                    tc.tile_pool(name="pzp", bufs=4, space="PSUM") as pzp,
                ):
                    for m in range(8, 16):
                        z_t = pzz.tile([128, L], F16, tag="zt", name="z_t")
                        for half in range(2):
                            for (off, n) in _nchunks(LH):
                                go = half * LH + off
                                ps = pzp.tile([128, 512], F32, tag="zmm", name="ps_z")
                                for k in range(4):
                                    nc.tensor.matmul(
                                        ps[:, 0:n],
                                        w1T_t[:, k, m * 128:(m + 1) * 128],
                                        hT_t[:, k, go:go + n],
                                        start=(k == 0), stop=(k == 3),
                                    )
                                nc.scalar.activation(z_t[:, go:go + n], ps[:, 0:n], AF.Silu)
                        nc.sync.dma_start(zbuf[m - 8, :, :], z_t[:])

            # ============ Phase D: conv branch ============
            with (
                tc.tile_pool(name="pd1", bufs=1) as pd1,
                tc.tile_pool(name="pd8", bufs=8) as pd8,
                tc.tile_pool(name="pd16", bufs=16) as pd16,
                tc.tile_pool(name="pdt", bufs=2) as pdt,
                tc.tile_pool(name="pdm", bufs=3, space="PSUM") as pdm,
                tc.tile_pool(name="pdc", bufs=3, space="PSUM") as pdc,
                tc.tile_pool(name="pdst", bufs=2, space="PSUM") as pdst,
            ):
                hTc_t = pd1.tile([128, 4, CEXT], F16, tag="hTc", name="hTc_t")
                for k in range(4):
                    nc.sync.dma_start(hTc_t[:, k, :], hTc[k * 128:(k + 1) * 128, :])
                wxz1T_t = pd1.tile([128, 4, DI], F16, tag="wxz1T", name="wxz1T_t")
                nc.sync.dma_start(wxz1T_t[:], wxz1T[:])
                mcombT_t = pd1.tile([128, 16, DM], F16, tag="mcombT", name="mcombT_t")
                nc.sync.dma_start(mcombT_t[:], mcombT[:])
                mgsumT_t = pd1.tile([1, DM], F16, tag="mgsumT", name="mgsumT_t")
                nc.sync.dma_start(mgsumT_t[:], mgsumT[:])
                dildiag_t = pd1.tile([128, 36, 128], F16, tag="dildiag", name="dildiag_t")
                nc.sync.dma_start(dildiag_t[:], dildiag[:])
                locdiag_t = pd1.tile([128, 12, 128], F16, tag="locdiag", name="locdiag_t")
                nc.sync.dma_start(locdiag_t[:], locdiag[:])
                dbias_t = pd1.tile([128, 12], F32, tag="dbias", name="dbias_t")
                nc.sync.dma_start(dbias_t[:], dbias[:])
                locb_t = pd1.tile([128, 4], F32, tag="locb", name="locb_t")
                nc.sync.dma_start(locb_t[:], locb[:])
                efixL_t = pd1.tile([128, 48], F16, tag="efixL", name="efixL_t")
                nc.sync.dma_start(efixL_t[:], efixL[:])
                efixR_t = pd1.tile([128, 48], F16, tag="efixR", name="efixR_t")
                nc.sync.dma_start(efixR_t[:], efixR[:])
                phi_t = pd1.tile([128, 4], F32, tag="phi", name="phi_t")
                nc.sync.dma_start(phi_t[:], phi_i[:])
                ones_t = pd1.tile([128, 1], F16, tag="ones", name="ones_t")
                nc.gpsimd.memset(ones_t[:], 1.0)
                one1f = pd1.tile([1, 128], F16, tag="one1f", name="one1f_t")
                nc.gpsimd.memset(one1f[:], 1.0)

                # xz1 = in_proj[4096:5120] @ hidden_window ; m 0-3: xa, 4-7: xc
                xz_tiles = []
                for m in range(8):
                    t = pd8.tile([128, CEXT], F16, tag="xz", name="xz_t")
                    for (off, n) in _nchunks(CEXT):
                        ps = pdm.tile([128, 512], F32, tag="dmm", name="ps_xz")
                        for k in range(4):
                            nc.tensor.matmul(
                                ps[:, 0:n],
                                wxz1T_t[:, k, m * 128:(m + 1) * 128],
                                hTc_t[:, k, off:off + n],
                                start=(k == 0), stop=(k == 3),
                            )
                        nc.scalar.copy(t[:, off:off + n], ps[:, 0:n])
                    xz_tiles.append(t)
                xa_tiles, xc_tiles = xz_tiles[:4], xz_tiles[4:]

                cat_tiles = []
                # feats: 3 dilations x 4 ch-tiles (cat channels 0..1535), PE conv
                for i, d in enumerate((1, 2, 4)):
                    for t4 in range(4):
                        ct = pd16.tile([128, LH], F16, tag="cat", name="ct_dil")
                        for (off, n) in _nchunks(LH):
                            cv = pdc.tile([128, 512], F32, tag="dcv", name="cv_dil")
                            for j in range(3):
                                o = 4 + (j - 1) * d + off
                                nc.tensor.matmul(
                                    cv[:, 0:n],
                                    dildiag_t[:, (i * 4 + t4) * 3 + j, :],
                                    xa_tiles[t4][:, o:o + n],
                                    start=(j == 0), stop=(j == 2),
                                )
                            nc.scalar.activation(ct[:, off:off + n], cv[:, 0:n], AF.Identity,
                                                 bias=dbias_t[:, i * 4 + t4:i * 4 + t4 + 1])
                        # boundary taps beyond the sequence: host-computed fixups
                        e = (i * 4 + t4) * 4
                        nc.vector.tensor_add(ct[:, 0:4], ct[:, 0:4], efixL_t[:, e:e + 4])
                        nc.vector.tensor_add(ct[:, LH - 4:LH], ct[:, LH - 4:LH],
                                             efixR_t[:, e:e + 4])
                        cat_tiles.append(ct)
                # phi * gelu(local conv + b)  (cat channels 1536..2047), PE conv
                for t4 in range(4):
                    lg = pdt.tile([128, LH], F16, tag="lg", name="lg_t")
                    for (off, n) in _nchunks(LH):
                        cv = pdc.tile([128, 512], F32, tag="dcv", name="cv_loc")
                        for j in range(3):
                            o = 3 + j + off
                            nc.tensor.matmul(
                                cv[:, 0:n],
                                locdiag_t[:, t4 * 3 + j, :],
                                xc_tiles[t4][:, o:o + n],
                                start=(j == 0), stop=(j == 2),
                            )
                        nc.scalar.activation(lg[:, off:off + n], cv[:, 0:n], AF.Gelu,
                                             bias=locb_t[:, t4:t4 + 1])
                    ct = pd16.tile([128, LH], F16, tag="cat", name="ct_loc")
                    nc.vector.tensor_scalar(ct[:], lg[:], phi_t[:, t4:t4 + 1], None, ALU.mult)
                    cat_tiles.append(ct)

                # LN folded into fuse matmul: need mu, rsd = 1/sqrt(var+eps)
                mu = pd1.tile([1, LH], F16, tag="mu", name="mu_t")
                ex2 = pd1.tile([1, LH], F32, tag="ex2", name="ex2_t")
                for (off, n) in _nchunks(LH):
                    pstat = pdst.tile([1, 512], F32, tag="st", name="pstat")
                    for t16 in range(16):
                        nc.tensor.matmul(pstat[0:1, 0:n], ones_t[:],
                                         cat_tiles[t16][:, off:off + n],
                                         start=(t16 == 0), stop=(t16 == 15),
                                         skip_group_check=True)
                    nc.scalar.activation(mu[:, off:off + n], pstat[0:1, 0:n],
                                         AF.Copy, scale=1.0 / 2048)
                for (off, n) in _nchunks(LH):
                    pstat2 = pdst.tile([1, 512], F32, tag="st", name="pstat2")
                    for t16 in range(16):
                        sq = pdt.tile([128, 512], F16, tag="sq", name="sq_t")
                        nc.vector.tensor_mul(sq[:, 0:n], cat_tiles[t16][:, off:off + n],
                                             cat_tiles[t16][:, off:off + n])
                        nc.tensor.matmul(pstat2[0:1, 0:n], ones_t[:], sq[:, 0:n],
                                         start=(t16 == 0), stop=(t16 == 15),
                                         skip_group_check=True)
                    nc.scalar.activation(ex2[:, off:off + n], pstat2[0:1, 0:n],
                                         AF.Copy, scale=1.0 / 2048)
                var = pd1.tile([1, LH], F32, tag="var", name="var_t")
                nc.vector.tensor_mul(var[:], mu[:], mu[:])
                nc.vector.tensor_sub(var[:], ex2[:], var[:])
                nc.vector.tensor_scalar_add(var[:], var[:], 1e-5)
                sd = pd1.tile([1, LH], F32, tag="sd", name="sd_t")
                nc.scalar.activation(sd[:], var[:], AF.Sqrt)
                rsd = pd1.tile([1, LH], F16, tag="rsd", name="rsd_t")
                with nc.allow_low_precision(reason="1/sd fits f16; tol 2e-2"):
                    nc.vector.reciprocal(rsd[:], sd[:])
                # replicate rsd to 128 partitions
                rsrep = pd1.tile([128, LH], F16, tag="rsrep", name="rsrep_t")
                for (off, n) in _nchunks(LH):
                    psr = pdm.tile([128, 512], F32, tag="dmm", name="ps_rs")
                    nc.tensor.matmul(psr[:, 0:n], one1f[:], rsd[:, off:off + n],
                                     start=True, stop=True)
                    nc.scalar.copy(rsrep[:, off:off + n], psr[:, 0:n])

                # fused (out_proj[:,2048:] @ cb_fuse_w @ diag(ln_g)) @ cat
                # with rank-1 mean correction, then * (1/sd) broadcast
                for m in range(4):
                    psf_sb = pdt.tile([128, LH], F16, tag="psfsb", name="psf_sb")
                    for (off, n) in _nchunks(LH):
                        psf = pdc.tile([128, 512], F32, tag="dcv", name="psf_t")
                        for k in range(16):
                            nc.tensor.matmul(
                                psf[:, 0:n],
                                mcombT_t[:, k, m * 128:(m + 1) * 128],
                                cat_tiles[k][:, off:off + n],
                                start=(k == 0), stop=False,
                            )
                        nc.tensor.matmul(
                            psf[:, 0:n],
                            mgsumT_t[0:1, m * 128:(m + 1) * 128],
                            mu[0:1, off:off + n],
                            start=False, stop=True,
                        )
                        nc.scalar.copy(psf_sb[:, off:off + n], psf[:, 0:n])
                    oc = pdt.tile([128, LH], F32, tag="oc", name="oc_t")
                    nc.vector.tensor_mul(oc[:], psf_sb[:], rsrep[:])
                    nc.sync.dma_start(o_conv[m * 128:(m + 1) * 128, :], oc[:])

            # ============ Phase C: selective scan ============
            with (
                tc.tile_pool(name="pca", bufs=1) as pca,
                tc.tile_pool(name="pq2", bufs=2) as pq2,
                tc.tile_pool(name="pq3", bufs=3) as pq3,
                tc.tile_pool(name="ps2p", bufs=2) as ps2p,
                tc.tile_pool(name="phl", bufs=8) as phl,
                tc.tile_pool(name="ppy", bufs=2, space="PSUM") as ppy,
                tc.tile_pool(name="pcp", bufs=2, space="PSUM") as pcp,
            ):
                hlast = [phl.tile([128, DS], F32, tag="hlast", name=f"hlast{i}")
                         for i in range(8)]
                yg_t = pca.tile([128, 8, LH], F16, tag="yg", name="yg_t")
                crep_all = pca.tile([128, DS * LH], F16, tag="crepall", name="crep_all")

                def emit_epilogue(c, psy, x_t, sz_t):
                    # yg = (y + D*x) * silu(z); psy -> SBUF so DVE runs 2x mode
                    psy_sb = ps2p.tile([128, LH], F16, tag="psysb", name="psy_sb")
                    nc.scalar.copy(psy_sb[:], psy[:])
                    tmp = ps2p.tile([128, LH], F16, tag="tmp", name="tmp_t")
                    nc.vector.scalar_tensor_tensor(tmp[:], x_t[:], Dsb_t[:, c:c + 1],
                                                   psy_sb[:], ALU.mult, ALU.add)
                    nc.vector.tensor_mul(yg_t[:, c, :], tmp[:], sz_t[:])

                def build_crep(n, off_h):
                    for (off, nn) in _nchunks(LH):
                        psc = pcp.tile([128, 512], F32, tag="cmm", name="ps_cr")
                        nc.tensor.matmul(
                            psc[:, 0:nn],
                            selT_t[:, (DS + n) * 128:(DS + n + 1) * 128],
                            xbc_sb[:, off_h + off:off_h + off + nn],
                            start=True, stop=True,
                        )
                        nc.scalar.copy(crep_all[:, n * LH + off:n * LH + off + nn],
                                       psc[:, 0:nn])

                for half in range(2):
                    off_h = half * LH
                    if half == 0:
                        for n in range(DS):
                            build_crep(n, 0)

                    prev = None
                    for c in range(8):
                        dl_t = ps2p.tile([128, LH], F16, tag="dls", name="dl_s")
                        nc.sync.dma_start(dl_t[:], dbuf[c, :, off_h:off_h + LH])
                        x_t = ps2p.tile([128, LH], F16, tag="xs", name="x_s")
                        nc.sync.dma_start(x_t[:], xbuf[c, :, off_h:off_h + LH])
                        sz_t = ps2p.tile([128, LH], F16, tag="szs", name="sz_s")
                        nc.sync.dma_start(sz_t[:], zbuf[c, :, off_h:off_h + LH])

                        psy = ppy.tile([128, LH], F32, tag="py", name="psy_t")
                        for qg in range(NQ):
                            dbu_q = pq3.tile([128, QL], F16, tag="dbu", name="dbu_q")
                            for j in range(Q):
                                nc.sync.dma_start(
                                    dbu_q[:, j * LH:(j + 1) * LH],
                                    dbu2[c, qg * Q + j, :, off_h:off_h + LH])
                            dA_q = pq2.tile([128, QL], F16, tag="dA", name="dA_q")
                            for j in range(Q):
                                nc.scalar.activation(
                                    dA_q[:, j * LH:(j + 1) * LH], dl_t[:], AF.Exp,
                                    scale=Asb_t[:, c * DS + qg * Q + j:c * DS + qg * Q + j + 1])
                            # state isolation at group-internal boundaries:
                            # zero dA first column (exact reset); for half 1
                            # inject the carried state via dbu first.
                            dA3 = dA_q[:].rearrange("p (q l) -> p q l", l=LH)
                            db3 = dbu_q[:].rearrange("p (q l) -> p q l", l=LH)
                            if half == 1:
                                pat = ps2p.tile([128, Q - 1], F16, tag="pat", name="pat_t")
                                hl = hlast[c][:, qg * Q + 1:qg * Q + Q].unsqueeze(2)
                                nc.vector.tensor_mul(pat[:].unsqueeze(2), dA3[:, 1:Q, 0:1], hl)
                                nc.vector.tensor_add(db3[:, 1:Q, 0:1], db3[:, 1:Q, 0:1],
                                                     pat[:].unsqueeze(2))
                            nc.vector.tensor_scalar(dA3[:, 1:Q, 0:1], dA3[:, 1:Q, 0:1],
                                                    0.0, None, ALU.mult)
                            init = 0.0 if half == 0 else hlast[c][:, qg * Q:qg * Q + 1]
                            hb_q = pq2.tile([128, QL], F16, tag="hb", name="hb_q")
                            nc.vector.tensor_tensor_scan(hb_q[:], dA_q[:], dbu_q[:], init,
                                                         ALU.mult, ALU.add)
                            if half == 0:
                                hb3 = hb_q[:].rearrange("p (q l) -> p q l", l=LH)
                                nc.vector.tensor_copy(
                                    hlast[c][:, qg * Q:(qg + 1) * Q].unsqueeze(2),
                                    hb3[:, :, LH - 1:LH])
                            hC_q = pq2.tile([128, QL], F16, tag="hC", name="hC_q")
                            nc.vector.tensor_mul(hC_q[:], hb_q[:],
                                                 crep_all[:, qg * QL:(qg + 1) * QL])
                            for j in range(Q):
                                for (off, nn) in _nchunks(LH):
                                    nc.tensor.matmul(
                                        psy[:, off:off + nn], id_t[:],
                                        hC_q[:, j * LH + off:j * LH + off + nn],
                                        start=(qg == 0 and j == 0),
                                        stop=(qg == NQ - 1 and j == Q - 1),
                                        skip_group_check=True)
                            if qg == 0 and prev is not None:
                                emit_epilogue(*prev)
                        # rebuild creps for half 1 while the last c's of half 0 run
                        if half == 0 and c == 7:
                            for n in range(DS):
                                build_crep(n, LH)
                        prev = (c, psy, x_t, sz_t)
                    emit_epilogue(*prev)

                    # out_proj partial for this half
                    for m in range(4):
                        pso = ppy.tile([128, LH], F32, tag="py", name="pso_t")
                        for (off, nn) in _nchunks(LH):
                            for c in range(8):
                                nc.tensor.matmul(
                                    pso[:, off:off + nn],
                                    wopT_t[:, c, m * 128:(m + 1) * 128],
                                    yg_t[:, c, off:off + nn],
                                    start=(c == 0), stop=(c == 7),
                                )
                        ot = ps2p.tile([128, LH], F32, tag="ot", name="ot_t")
                        nc.scalar.copy(ot[:], pso[:])
                        nc.sync.dma_start(o_scan[m * 128:(m + 1) * 128, off_h:off_h + LH], ot[:])

    split_sync_waits(nc)
    return nc


_CACHE = {}


def _get_nc():
    if "nc" not in _CACHE:
        _CACHE["nc"] = build_nc()
    return _CACHE["nc"]


def _prep_in_maps(inputs):
    f16, f32 = np.float16, np.float32
    hidden = np.asarray(inputs["hidden_states"], f32)      # (B, L, DM)
    in_proj_w = np.asarray(inputs["in_proj_w"], f32)       # (5120, 512)
    conv1d_w = np.asarray(inputs["conv1d_w"], f32)         # (DI, 1, 4)
    conv1d_b = np.asarray(inputs["conv1d_b"], f32)
    x_proj_w = np.asarray(inputs["x_proj_w"], f32)         # (64, DI)
    dt_proj_w = np.asarray(inputs["dt_proj_w"], f32)       # (DI, 32)
    dt_proj_b = np.asarray(inputs["dt_proj_b"], f32)
    A = -np.exp(np.asarray(inputs["A_log"], f32))          # (DI, DS)
    D = np.asarray(inputs["D"], f32)
    out_proj_w = np.asarray(inputs["out_proj_w"], f32)     # (512, 3072)
    cb_local_w = np.asarray(inputs["cb_local_w"], f32)     # (512,1,3)
    cb_local_b = np.asarray(inputs["cb_local_b"], f32)
    cb_global_w = np.asarray(inputs["cb_global_w"], f32)   # (512,1,1)
    cb_global_b = np.asarray(inputs["cb_global_b"], f32)
    cb_pre_w = np.asarray(inputs["cb_pre_w"], f32)         # (3,512,1,1)
    cb_pre_b = np.asarray(inputs["cb_pre_b"], f32)         # (3,512)
    cb_dil_w = np.asarray(inputs["cb_dil_w"], f32)         # (3,512,1,3)
    cb_dil_b = np.asarray(inputs["cb_dil_b"], f32)
    cb_ln_g = np.asarray(inputs["cb_ln_g"], f32)           # (2048,)
    cb_ln_b = np.asarray(inputs["cb_ln_b"], f32)
    cb_fuse_w = np.asarray(inputs["cb_fuse_w"], f32)       # (1024, 2048, 1)
    cb_fuse_b = np.asarray(inputs["cb_fuse_b"], f32)

    # host precomputes
    Wop3 = out_proj_w[:, 2 * DI:]                                  # (512, 1024)
    M = Wop3 @ cb_fuse_w[:, :, 0]                                  # (512, 2048)
    Mg = M * cb_ln_g[None, :]                                      # ln gain folded
    cbias_vec = Wop3 @ cb_fuse_b + M @ cb_ln_b                     # (512,)
    mgsum = Mg.sum(axis=1)                                         # (512,)
    hmean = hidden.mean(axis=1)                                    # (B, 512)
    W_xc = in_proj_w[4 * DI + DM:4 * DI + 2 * DM]                  # (512, 512) -> xc rows
    xcm_mean = hmean @ W_xc.T                                      # (B, 512)
    phi = np.maximum(cb_global_w[:, 0, 0][None, :] * xcm_mean + cb_global_b[None, :], 0.0)

    def lhsT3(w, kdim=128):  # (K, M) -> (128, K//128, M)
        K, Mm = w.shape
        return np.ascontiguousarray(w.reshape(K // kdim, kdim, Mm).transpose(1, 0, 2))

    def perpart(v):  # (n*128,) -> (128, n)
        return np.ascontiguousarray(v.reshape(-1, 128).T)

    def diagpack(vals):  # (nmat, 128) -> (128, nmat, 128) diagonal lhsT mats
        nmat = vals.shape[0]
        out = np.zeros((128, nmat, 128), f16)
        idx = np.arange(128)
        for i in range(nmat):
            out[idx, i, idx] = vals[i]
        return out

    selT = np.zeros((32, 2 * DS * 128), f16)
    for n in range(DS):
        selT[n, n * 128:(n + 1) * 128] = 1.0
        selT[DS + n, DS * 128 + n * 128:DS * 128 + (n + 1) * 128] = 1.0

    cw2 = conv1d_w[:, 0, :].reshape(8, 128, 4)
    cvals = np.ascontiguousarray(cw2.transpose(0, 2, 1).reshape(32, 128))
    dw = cb_dil_w[:, :, 0, :]                                      # (3, 512, 3)
    pw = cb_pre_w[:, :, 0, 0]                                      # (3, 512)
    dvals = np.zeros((36, 128), f32)
    for i in range(3):
        for t4 in range(4):
            for j in range(3):
                ch = slice(t4 * 128, (t4 + 1) * 128)
                dvals[(i * 4 + t4) * 3 + j] = dw[i, ch, j] * pw[i, ch]
    lvals = np.zeros((12, 128), f32)
    for t4 in range(4):
        for j in range(3):
            lvals[t4 * 3 + j] = cb_local_w[t4 * 128:(t4 + 1) * 128, 0, j]
    dbias_m = np.zeros((128, 12), f32)
    for i in range(3):
        for t4 in range(4):
            ch = slice(t4 * 128, (t4 + 1) * 128)
            dbias_m[:, i * 4 + t4] = (cb_pre_b[i, ch] * dw[i, ch, :].sum(-1)
                                      + cb_dil_b[i, ch])

    common = dict(
        cbias=perpart(conv1d_b),
        xpT=lhsT3(x_proj_w.T).astype(f16),
        selT=selT,
        dpT=np.ascontiguousarray(dt_proj_w.T).astype(f16),
        dpb=perpart(dt_proj_b),
        Asb=np.ascontiguousarray(A.reshape(8, 128, DS).transpose(1, 0, 2).reshape(128, 128)),
        Dsb=perpart(D),
        ident=np.eye(128, dtype=f16),
        cdiag=diagpack(cvals),
        dildiag=diagpack(dvals),
        locdiag=diagpack(lvals),
        dbias=dbias_m,
        locb=perpart(cb_local_b),
        mcombT=lhsT3(Mg.T).astype(f16),
        mgsumT=np.ascontiguousarray(-mgsum[None, :]).astype(f16),
        wxz1T=lhsT3(in_proj_w[4 * DI:].T).astype(f16),
    )
    common = {k: np.ascontiguousarray(v) for k, v in common.items()}

    in_maps = []
    for c in range(NC8):
        b, dirn = c % 4, c // 4
        bc, halfc = c // 2, c % 2
        hT_b = hidden[b].T                                  # (512, L)
        if dirn == 1:
            hT_b = hT_b[:, ::-1]
        W1 = in_proj_w[dirn * 2 * DI:(dirn + 1) * 2 * DI]   # (2048, 512)
        Wop = out_proj_w[:, dirn * DI:(dirn + 1) * DI]      # (512, 1024)
        # conv window [start-4, end+4) zero-padded outside [0, L)
        s0 = halfc * LH - 4
        win = np.zeros((DM, CEXT), f32)
        lo, hi = max(s0, 0), min(s0 + CEXT, L)
        win[:, lo - s0:hi - s0] = hidden[bc].T[:, lo:hi]
        # edge fixups for the dilated-conv preb term: columns where a tap
        # falls outside [0, L).  halfc=0 -> left edge (t < d, j=0 tap),
        # halfc=1 -> right edge (t >= LH-d, j=2 tap).  value = -preb*dilw_j
        efl = np.zeros((128, 48), f16)
        efr = np.zeros((128, 48), f16)
        for i, dd in enumerate((1, 2, 4)):
            for t4 in range(4):
                ch = slice(t4 * 128, (t4 + 1) * 128)
                e = (i * 4 + t4) * 4
                if halfc == 0:
                    for t in range(min(dd, 4)):
                        efl[:, e + t] = -(cb_pre_b[i, ch] * dw[i, ch, 0])
                else:
                    for tt in range(4):
                        if tt >= 4 - dd:
                            efr[:, e + tt] = -(cb_pre_b[i, ch] * dw[i, ch, 2])
        in_maps.append(dict(
            common,
            hT=hT_b.astype(f16),
            hTc=win.astype(f16),
            w1T=lhsT3(W1.T).astype(f16),
            wopT=lhsT3(Wop.T).astype(f16),
            phi_i=perpart(phi[bc]),
            efixL=efl,
            efixR=efr,
        ))
    in_maps = [{k: np.ascontiguousarray(v) for k, v in m.items()} for m in in_maps]
    return in_maps, cbias_vec


def _assemble(results, cbias_vec):
    out = np.zeros((B, L, DM), np.float32)
    for c in range(NC8):
        b, dirn = c % 4, c // 4
        bc, halfc = c // 2, c % 2
        oscan = results[c]["o_scan"]          # (512, L)
        if dirn == 1:
            oscan = oscan[:, ::-1]
        out[b] += oscan.T
        out[bc, halfc * LH:(halfc + 1) * LH] += results[c]["o_conv"].T
    out += cbias_vec[None, None, :]
    return out


def kernel(**inputs):
    nc = _get_nc()
    in_maps, cbias_vec = _prep_in_maps(inputs)
    res = run_bass_kernel_spmd(nc, in_maps, list(range(NC8)))
    return _assemble(res.results, cbias_vec)
